# revision 48
# baseline (speedup 1.0000x reference)
"""Single transformer block on 8 NeuronCores — collective-free.

Sharding: core c = (batch b=c//2, parity p=c%2). Each core receives the FULL
sequence of its batch, permuted to [own-stripe | peer-stripe] order, and
recomputes K and V for all 2048 tokens locally — cheaper than the pairwise
AllGather it replaces (~55us extra PE vs ~270us of collective time) and it
deletes all DRAM bounce traffic.  Q / attention / c_proj / MLP cover only the
core's 1024 own (striped) tokens.

Tricks:
  - LayerNorm affine (w, b) folds host-side into the following matmul
    weights/bias, so on-chip LN is just (x - mean) * rsqrt(var + eps).
  - The 1/sqrt(hd) query scale folds host-side into W_q / b_q.
  - V is built directly in [128 key, KC, H, 65] layout with a ones column at
    65, so AV yields the softmax denominator for free and per-head V slices
    are zero-copy views.
  - Scores are computed transposed S^T[k, q]; causal mask is a 0/1 multiply
    on P = exp(S) (finite, exact).  A mask differs from all-ones only in the
    chunk's diagonal 128-query slot, so masks are [128, 128] and cheap.
  - AV accumulates a whole 512-query group into one [65, 512] PSUM bank;
    a key chunk whose minimal covered slot is s_min only runs over query
    columns >= s_min*BS, so late chunks run narrow (128-token stripes keep
    the causal waste small and both parities balanced).
  - fp8 (e4m3): QKV projections and mproj run DoubleRow matmuls (2x PE
    rate, half the instructions; weights pre-scaled x64 host-side to stay
    out of the fp8 subnormal range, undone at eviction).  P and V are fp8
    too (scores have std ~0.4, so P <= e^2.5 fits easily), which makes AV a
    DoubleRow over (own, peer) chunk pairs.  fc stays bf16: quantizing it
    pushed rel err past the gate (2.4e-2); this config measures 1.68e-2.
  - Both heads of a 128-feature block run as interleaved QK->exp->AV chains
    to hide cross-engine latency.
"""

import math
from contextlib import ExitStack

import numpy as np
import ml_dtypes

import concourse.bacc as bacc
import concourse.bass as bass
import concourse.mybir as mybir
import concourse.tile as tile
from concourse.masks import make_identity

F32 = mybir.dt.float32
F32R = mybir.dt.float32r
BF16 = mybir.dt.bfloat16
F8 = mybir.dt.float8e4
AF = mybir.ActivationFunctionType
ALU = mybir.AluOpType

EPS = 1e-5


class Cfg:
    def __init__(self, B=4, S=2048, D=1024, H=16, F=4096, n_cores=8,
                 qkv_fp8=True, fc_fp8=False, mproj_fp8=True, av_fp8=True,
                 qk_fp8=False, BS=128):
        self.B, self.S, self.D, self.H, self.F = B, S, D, H, F
        self.n_cores = n_cores
        assert n_cores == 2 * B
        self.HD = D // H
        assert self.HD == 64
        self.T = S // 2            # tokens owned per core
        self.KT = S // 128         # token 128-tiles, full sequence
        self.TB = self.T // 128    # token 128-tiles, local
        self.DC = D // 128         # contraction chunks over D
        self.QF = 512              # free-dim tile for projection matmuls
        self.KC = S // 128         # key 128-chunks over full sequence
        self.GB = F // 128         # MLP hidden 128-blocks
        self.HPB = 128 // self.HD  # heads per 128-feature block (=2)
        self.BS = BS               # stripe block (q-slot) size
        self.SLOTS = self.T // self.BS
        self.SPG = 512 // self.BS  # slots per 512-wide attention group
        self.KCH = self.KC // 2    # chunks per parity half
        self.CPB = self.BS // 128  # key chunks per stripe block
        self.qkv_fp8 = qkv_fp8
        self.fc_fp8 = fc_fp8
        self.mproj_fp8 = mproj_fp8
        self.av_fp8 = av_fp8
        self.qk_fp8 = qk_fp8
        self.wscale = 64.0 if qkv_fp8 else 1.0
        self.fscale = 64.0 if fc_fp8 else 1.0
        self.mscale = 64.0 if mproj_fp8 else 1.0


def build(cfg: Cfg):
    c = cfg
    QT = F8 if c.qkv_fp8 else BF16
    FT = F8 if c.fc_fp8 else BF16
    MPT = F8 if c.mproj_fp8 else BF16
    VT = F8 if c.av_fp8 else BF16
    PT = VT
    KQT = F8 if c.qk_fp8 else BF16
    nc = bacc.Bacc(None, target_bir_lowering=False)

    # ---------------- I/O ----------------
    x_in = nc.dram_tensor("x", [c.S, c.D], F32, kind="ExternalInput")
    w_attn = nc.dram_tensor("w_attn", [c.D, 3 * c.D], QT, kind="ExternalInput")
    w_cproj = nc.dram_tensor("w_cproj", [c.D, c.D], BF16, kind="ExternalInput")
    w_fc = nc.dram_tensor("w_fc", [c.D, c.F], FT, kind="ExternalInput")
    w_mproj = nc.dram_tensor("w_mproj", [c.F, c.D], MPT,
                             kind="ExternalInput")
    battn_qk_in = nc.dram_tensor("battn_qk", [128, 2 * c.DC], F32,
                                 kind="ExternalInput")
    bv_in = nc.dram_tensor("bv", [1, c.D], F32, kind="ExternalInput")
    bcp_in = nc.dram_tensor("bcp", [1, c.D], F32, kind="ExternalInput")
    bmp_in = nc.dram_tensor("bmp", [1, c.D], F32, kind="ExternalInput")
    bfc_in = nc.dram_tensor("bfc", [128, c.GB], F32, kind="ExternalInput")
    qidx_in = nc.dram_tensor("qidx", [1, c.T], F32, kind="ExternalInput")
    kofs_in = nc.dram_tensor("kofs", [128, c.KC], F32, kind="ExternalInput")
    y_out = nc.dram_tensor("y", [c.T, c.D], F32, kind="ExternalOutput")

    def bcast(dram, p=128):
        # partition-broadcast DMA source: read row 0 for every partition
        return bass.AP(tensor=dram, offset=0, ap=[[0, p], [1, dram.shape[1]]])

    with tile.TileContext(nc) as tc, ExitStack() as es:
        gconst = es.enter_context(tc.tile_pool(name="gconst", bufs=1))
        ident = gconst.tile([128, 128], F32)
        make_identity(nc, ident[:])
        eps_t = gconst.tile([128, 1], F32)
        nc.vector.memset(eps_t[:], EPS)
        ones64_f = gconst.tile([1, 64], F32)
        nc.vector.memset(ones64_f[:], 1.0)
        ones64 = gconst.tile([1, 64], F32R)
        nc.vector.tensor_copy(ones64[:], ones64_f[:])

        def layernorm_to(get_src, n_tiles, dest, lnp, ps_tr, tag,
                         interleave=None):
            """normalize token tiles and write feature-major into dest
            [128, DC, n_tiles*128].  get_src(tb) -> token-major [128, D] tile.
            interleave(g) is called after every 4th tile to emit consumer
            work early (keeps PE fed in emission order)."""
            for tb in range(n_tiles):
                src = get_src(tb)
                st = lnp.tile([128, 2, 6], F32, tag=f"{tag}st")
                for sg in range(2):
                    nc.vector.bn_stats(
                        out=st[:, sg, :], in_=src[:, sg * 512:(sg + 1) * 512])
                mv = lnp.tile([128, 2], F32, tag=f"{tag}mv")
                nc.vector.bn_aggr(out=mv[:], in_=st[:])
                sd = lnp.tile([128, 1], F32, tag=f"{tag}sd")
                nc.scalar.activation(sd[:], mv[:, 1:2], AF.Sqrt,
                                     bias=eps_t[:, 0:1])
                rs = lnp.tile([128, 1], F32, tag=f"{tag}rs")
                nc.vector.reciprocal(rs[:], sd[:])
                nrm = lnp.tile([128, c.D], F32, tag=f"{tag}n")
                nc.vector.tensor_scalar(
                    out=nrm[:], in0=src[:], scalar1=mv[:, 0:1],
                    scalar2=rs[:, 0:1], op0=ALU.subtract, op1=ALU.mult)
                for i2 in range(c.DC // 4):
                    pt = ps_tr.tile([128, 512], F32, tag=f"{tag}tr")
                    for j in range(4):
                        ch = 4 * i2 + j
                        nc.tensor.matmul(
                            pt[:, j * 128:(j + 1) * 128],
                            nrm[:, ch * 128:(ch + 1) * 128], ident[:],
                            is_transpose=True, start=(j == 0), stop=(j == 3))
                    nc.scalar.activation(
                        dest[:, 4 * i2:4 * i2 + 4, tb * 128:(tb + 1) * 128],
                        pt[:], AF.Identity)
                if interleave is not None and tb % 4 == 3:
                    interleave(tb // 4)

        # ---------------- persistent activations ----------------
        es_per = ExitStack()
        xloc = []
        xlp = es_per.enter_context(tc.tile_pool(name="xloc", bufs=1,
                                                side="left"))
        xq = [nc.sync, nc.gpsimd]
        for tb in range(c.TB):
            t = xlp.tile([128, c.D], F32, tag=f"x{tb}", name=f"x{tb}")
            xq[tb % 2].dma_start(out=t[:],
                                 in_=x_in[tb * 128:(tb + 1) * 128, :])
            xloc.append(t)

        es_kvq = ExitStack()
        kvqp = es_kvq.enter_context(tc.tile_pool(name="kvq", bufs=1,
                                                 side="right"))
        ktp = kvqp.tile([128, c.DC, c.S], KQT, name="ktp")
        vtt = kvqp.tile([128, 2, c.KCH, c.H, 65], VT, name="vtt")
        qtp = kvqp.tile([128, c.DC, c.T], KQT, name="qtp")
        nc.vector.memset(vtt[:, :, :, :, 64:65], 1.0)

        # ================= phase A: LN1 + QKV =================
        es_ht = ExitStack()
        htp = es_ht.enter_context(tc.tile_pool(name="htp", bufs=1))
        ht = htp.tile([128, c.DC, c.S], QT, name="ht")

        with (
            tc.tile_pool(name="aconst", bufs=1) as aconst,
            tc.tile_pool(name="xs", bufs=2) as xsp,
            tc.tile_pool(name="lnp", bufs=2) as lnp,
            tc.tile_pool(name="wa", bufs=2) as wap,
            tc.tile_pool(name="ko", bufs=4) as kop,
            tc.tile_pool(name="ps_tr", bufs=2, space="PSUM") as ps_tr,
            tc.tile_pool(name="ps_mm", bufs=2, space="PSUM") as psmm,
        ):
            battn_qk = aconst.tile([128, 2 * c.DC], F32)
            nc.sync.dma_start(out=battn_qk[:], in_=battn_qk_in[:, :])
            bv_b = aconst.tile([128, c.D], F32)
            nc.sync.dma_start(out=bv_b[:], in_=bcast(bv_in))
            bcp_b = aconst.tile([128, c.D], F32)
            nc.sync.dma_start(out=bcp_b[:], in_=bcast(bcp_in))

            wk = wap.tile([128, c.DC, c.D], QT, tag="wa", name="wk")
            nc.scalar.dma_start(
                out=wk[:],
                in_=w_attn[:, c.D:2 * c.D].rearrange("(i p) f -> p i f",
                                                     p=128))
            wv = wap.tile([128, c.DC, c.D], QT, tag="wa", name="wv")
            nc.scalar.dma_start(
                out=wv[:],
                in_=w_attn[:, 2 * c.D:3 * c.D].rearrange("(i p) f -> p i f",
                                                         p=128))

            inv_w = 1.0 / c.wscale
            NI = c.DC // 2 if c.qkv_fp8 else c.DC  # contraction steps

            def wsl(wslab, i, fsl):
                # weight slab contraction-step slice (pair of chunks in fp8)
                if c.qkv_fp8:
                    return wslab[:, 2 * i:2 * i + 2, fsl]
                return wslab[:, i, fsl]

            def hsl(i, tsl):
                if c.qkv_fp8:
                    return ht[:, 2 * i:2 * i + 2, tsl]
                return ht[:, i, tsl]

            PM = mybir.MatmulPerfMode.DoubleRow if c.qkv_fp8 else None

            def v_chunk(g):
                # V for these 4 token tiles; lhsT (ht slice) shared across
                # both feature halves so ldweights is loaded once per step
                for tb in range(4 * g, 4 * g + 4):
                    tbs = slice(tb * 128, (tb + 1) * 128)
                    pss = [psmm.tile([128, 512], F32, tag="ps", name=f"psv{q}")
                           for q in range(2)]
                    for i in range(NI):
                        for vh in range(2):
                            nc.tensor.matmul(
                                pss[vh][:], hsl(i, tbs),
                                wsl(wv, i, slice(vh * 512, (vh + 1) * 512)),
                                start=(i == 0), stop=(i == NI - 1),
                                perf_mode=PM)
                    for vh in range(2):
                        fsl = slice(vh * 512, (vh + 1) * 512)
                        dst = vtt[:, tb // c.KCH, tb % c.KCH,
                                  vh * 8:(vh + 1) * 8, 0:64]
                        if c.qkv_fp8:
                            vo = kop.tile([128, 512], BF16, tag="vo")
                            nc.scalar.activation(vo[:], pss[vh][:],
                                                 AF.Identity, scale=inv_w)
                            nc.vector.tensor_add(dst, vo[:], bv_b[:, fsl])
                        else:
                            nc.vector.tensor_add(dst, pss[vh][:],
                                                 bv_b[:, fsl])

            def get_src(tb):
                if tb < c.TB:
                    return xloc[tb]
                t = xsp.tile([128, c.D], F32, tag="xs")
                xq[tb % 2].dma_start(out=t[:],
                                     in_=x_in[tb * 128:(tb + 1) * 128, :])
                return t

            layernorm_to(get_src, c.KT, ht, lnp, ps_tr, "a",
                         interleave=v_chunk)

            # ---- k^T pass: lhsT (w chunk) shared across 4 token slices ----
            for m in range(c.DC):
                msl = slice(m * 128, (m + 1) * 128)
                pss = [psmm.tile([128, 2, 512], F32, tag="pk",
                                 name=f"psk{q}") for q in range(2)]
                for i in range(NI):
                    for th in range(4):
                        nc.tensor.matmul(
                            pss[th // 2][:, th % 2, :], wsl(wk, i, msl),
                            hsl(i, slice(th * 512, (th + 1) * 512)),
                            start=(i == 0), stop=(i == NI - 1),
                            perf_mode=PM)
                for q in range(2):
                    nc.scalar.activation(
                        ktp[:, m, q * 1024:(q + 1) * 1024], pss[q][:],
                        AF.Identity,
                        bias=battn_qk[:, c.DC + m:c.DC + m + 1], scale=inv_w)

            # ---- q^T pass (local tokens only; scale folded host-side) ----
            wq = wap.tile([128, c.DC, c.D], QT, tag="wa", name="wq")
            nc.scalar.dma_start(
                out=wq[:],
                in_=w_attn[:, 0:c.D].rearrange("(i p) f -> p i f", p=128))
            for m in range(c.DC):
                msl = slice(m * 128, (m + 1) * 128)
                psq = psmm.tile([128, 2, 512], F32, tag="pk", name="psq")
                for i in range(NI):
                    for th in range(2):
                        nc.tensor.matmul(
                            psq[:, th, :], wsl(wq, i, msl),
                            hsl(i, slice(th * 512, (th + 1) * 512)),
                            start=(i == 0), stop=(i == NI - 1),
                            perf_mode=PM)
                nc.scalar.activation(
                    qtp[:, m, :], psq[:],
                    AF.Identity, bias=battn_qk[:, m:m + 1], scale=inv_w)

            # fold the c_proj bias into the residual copy of x, in place
            for tb in range(c.TB):
                nc.vector.tensor_add(xloc[tb][:], xloc[tb][:], bcp_b[:])

        es_ht.close()

        # ================= phase B: attention =================
        # prefetch c_proj weights during attention
        es_wc = ExitStack()
        wcp = es_wc.enter_context(tc.tile_pool(name="wc", bufs=1,
                                               side="left"))
        wc = wcp.tile([128, c.DC, c.D], BF16, name="wc")
        nc.scalar.dma_start(
            out=wc[:], in_=w_cproj[:, :].rearrange("(i p) f -> p i f", p=128))

        es_at = ExitStack()
        atp = es_at.enter_context(tc.tile_pool(name="atp", bufs=1,
                                               side="left"))
        at = atp.tile([128, c.DC, c.T], BF16, name="at")

        with (
            tc.tile_pool(name="bconst", bufs=1) as bconst,
            tc.tile_pool(name="mask", bufs=1) as maskp,
            tc.tile_pool(name="pt", bufs=4) as ptp,
            tc.tile_pool(name="rec", bufs=3) as recp,
            tc.tile_pool(name="ps_qk", bufs=2, space="PSUM") as psqk,
            tc.tile_pool(name="ps_o", bufs=3, space="PSUM") as pso,
            tc.tile_pool(name="ps_bc", bufs=1, space="PSUM") as psbc,
        ):
            qidx = bconst.tile([128, c.T], F32)
            nc.sync.dma_start(out=qidx[:], in_=bcast(qidx_in))
            kofs = bconst.tile([128, c.KC], F32)
            nc.sync.dma_start(out=kofs[:], in_=kofs_in[:, :])

            groups = [list(range(c.SPG * gi, c.SPG * (gi + 1)))
                      for gi in range(c.SLOTS // c.SPG)]

            # A chunk's mask differs from all-ones only in the diagonal
            # slot s_min (queries in later slots are >= every key of the
            # chunk for either parity), so one [128, BS] mask per chunk.
            masks = {}
            for kc in range(c.KC):
                s_min = (kc % c.KCH) // c.CPB
                qsl = slice(s_min * c.BS, (s_min + 1) * c.BS)
                mk = maskp.tile([128, c.BS], BF16, tag=f"mk{kc}",
                                name=f"mk{kc}")
                nc.vector.tensor_scalar(
                    out=mk[:], in0=qidx[:, qsl],
                    scalar1=kofs[:, kc:kc + 1], scalar2=None,
                    op0=ALU.is_ge)
                masks[kc] = mk

            # both heads of a feature block run interleaved: two independent
            # QK->exp->AV chains hide each other's cross-engine latencies
            for jj in range(c.DC):
                for gi, g in enumerate(groups):
                    s0, s3 = g[0], g[-1]
                    n_loc = (s3 + 1) * c.CPB
                    pos = [pso.tile([65, 512], F32, tag="po",
                                    name=f"po{hp}") for hp in range(c.HPB)]
                    for loc in range(n_loc):
                        lo = max(s0, loc // c.CPB)
                        w = (s3 - lo + 1) * c.BS
                        ocol = (lo - s0) * c.BS
                        qsl = slice(lo * c.BS, (s3 + 1) * c.BS)
                        for hp in range(c.HPB):
                            h = c.HPB * jj + hp
                            base = hp * 64
                            ps2 = psqk.tile([128, 2, 512], F32, tag="qk")
                            pt = ptp.tile([128, 2, 512], PT, tag="pt")
                            for ix in range(2):
                                kc = loc + ix * c.KCH
                                nc.tensor.matmul(
                                    ps2[:, ix, 0:w],
                                    ktp[base:base + 64, jj,
                                        kc * 128:(kc + 1) * 128],
                                    qtp[base:base + 64, jj, qsl],
                                    start=True, stop=True)
                            nc.scalar.activation(pt[:, :, 0:w],
                                                 ps2[:, :, 0:w], AF.Exp)
                            if loc // c.CPB >= s0:
                                for ix in range(2):
                                    kc = loc + ix * c.KCH
                                    nc.vector.tensor_mul(
                                        pt[:, ix, 0:c.BS],
                                        pt[:, ix, 0:c.BS], masks[kc][:])
                            if c.av_fp8:
                                nc.tensor.matmul(
                                    pos[hp][:, ocol:512],
                                    vtt[:, :, loc, h, :], pt[:, :, 0:w],
                                    start=(loc == 0),
                                    stop=(loc == n_loc - 1),
                                    perf_mode=mybir.MatmulPerfMode.DoubleRow)
                            else:
                                for ix in range(2):
                                    nc.tensor.matmul(
                                        pos[hp][:, ocol:512],
                                        vtt[:, ix, loc, h, :],
                                        pt[:, ix, 0:w],
                                        start=(loc == 0 and ix == 0),
                                        stop=(loc == n_loc - 1 and ix == 1))
                    # normalize by softmax denominator (row 64)
                    gq = slice(s0 * c.BS, s0 * c.BS + 512)
                    for hp in range(c.HPB):
                        base = hp * 64
                        po = pos[hp]
                        rec = recp.tile([1, 512], F32R, tag="rec")
                        with nc.allow_low_precision(
                                reason="softmax denom in f32r"):
                            nc.vector.reciprocal(rec[:], po[64:65, :])
                        bc = psbc.tile([64, 512], F32, tag="bc")
                        nc.tensor.matmul(bc[:], ones64[:], rec[:],
                                         start=True, stop=True)
                        bcs = recp.tile([64, 512], F32, tag="bcs")
                        nc.vector.tensor_copy(bcs[:], bc[:])
                        nc.vector.tensor_mul(
                            at[base:base + 64, jj, gq], po[0:64, :], bcs[:])

        es_kvq.close()

        # ================= phase C: c_proj + residual =================
        es_x2 = ExitStack()
        x2p = es_x2.enter_context(tc.tile_pool(name="x2p", bufs=1,
                                               side="right"))
        x2t = []
        with tc.tile_pool(name="ps_c", bufs=4, space="PSUM") as psc:
            for tb in range(c.TB):
                x2 = x2p.tile([128, c.D], F32, tag=f"x2_{tb}",
                              name=f"x2_{tb}")
                pss = [psc.tile([128, 512], F32, tag="ps", name=f"psc{q}") for q in range(2)]
                for i in range(c.DC):
                    for fh in range(2):
                        nc.tensor.matmul(
                            pss[fh][:], at[:, i, tb * 128:(tb + 1) * 128],
                            wc[:, i, fh * 512:(fh + 1) * 512],
                            start=(i == 0), stop=(i == c.DC - 1))
                for fh in range(2):
                    fsl = slice(fh * 512, (fh + 1) * 512)
                    nc.vector.tensor_add(x2[:, fsl], pss[fh][:],
                                         xloc[tb][:, fsl])
                x2t.append(x2)

        es_at.close()
        es_wc.close()
        es_per.close()

        # ================= phase D: LN2 + MLP =================
        with (
            tc.tile_pool(name="dconst", bufs=1) as dconst,
            tc.tile_pool(name="gt", bufs=1) as gtp,
            tc.tile_pool(name="wm", bufs=1) as wmp,
        ):
            bmp_b = dconst.tile([128, c.D], F32)
            nc.sync.dma_start(out=bmp_b[:], in_=bcast(bmp_in))
            bfc = dconst.tile([128, c.GB], F32)
            nc.sync.dma_start(out=bfc[:], in_=bfc_in[:, :])

            PMf = mybir.MatmulPerfMode.DoubleRow if c.fc_fp8 else None
            PMm = mybir.MatmulPerfMode.DoubleRow if c.mproj_fp8 else None
            NI2 = c.DC // 2 if c.fc_fp8 else c.DC    # fc contraction steps
            NG = c.GB // 2 if c.mproj_fp8 else c.GB  # mproj contraction steps
            inv_f = 1.0 / c.fscale
            inv_m = 1.0 / c.mscale
            gt = gtp.tile([128, c.GB, c.T], MPT, name="gt")
            x2b = [gtp.tile([128, c.D], F32, tag=f"x2b{tb}",
                            name=f"x2b{tb}") for tb in range(c.TB)]
            # mproj weights prefetched during LN2/fc on the sync queue so
            # they don't serialize behind the fc slab loads (scalar queue)
            wm_all = []
            for fh in range(2):
                wm = wmp.tile([128, c.GB, 512], MPT, tag=f"wm{fh}",
                              name=f"wm{fh}")
                nc.sync.dma_start(
                    out=wm[:],
                    in_=w_mproj[:, fh * 512:(fh + 1) * 512].rearrange(
                        "(g p) f -> p g f", p=128))
                wm_all.append(wm)
            with (
                tc.tile_pool(name="mtp", bufs=1) as mtp,
                tc.tile_pool(name="lnp2", bufs=2) as lnp2,
                tc.tile_pool(name="wf", bufs=2) as wfp,
                tc.tile_pool(name="ps_tr2", bufs=3, space="PSUM") as ps_tr2,
                tc.tile_pool(name="ps_g", bufs=2, space="PSUM") as psg,
            ):
                mt = mtp.tile([128, c.DC, c.T], FT, name="mt")
                layernorm_to(lambda tb: x2t[tb], c.TB, mt, lnp2, ps_tr2, "d")
                for tb in range(c.TB):
                    nc.vector.tensor_add(x2b[tb][:], x2t[tb][:], bmp_b[:])

                def msl(i, tsl):
                    if c.fc_fp8:
                        return mt[:, 2 * i:2 * i + 2, tsl]
                    return mt[:, i, tsl]

                # ---------------- fc + gelu ----------------
                wf = None
                for gb in range(c.GB):
                    if gb % 4 == 0:
                        wf = wfp.tile([128, c.DC, 512], FT, tag="wf",
                                      name=f"wf{gb}")
                        j = gb // 4
                        nc.scalar.dma_start(
                            out=wf[:],
                            in_=w_fc[:, j * 512:(j + 1) * 512].rearrange(
                                "(i p) f -> p i f", p=128))
                    gl = (gb % 4) * 128
                    ps = psg.tile([128, 1024], F32, tag="ps")
                    for i in range(NI2):
                        wfs = (wf[:, 2 * i:2 * i + 2, gl:gl + 128]
                               if c.fc_fp8 else wf[:, i, gl:gl + 128])
                        for th in range(2):
                            nc.tensor.matmul(
                                ps[:, th * 512:(th + 1) * 512], wfs,
                                msl(i, slice(th * 512, (th + 1) * 512)),
                                start=(i == 0), stop=(i == NI2 - 1),
                                perf_mode=PMf)
                    nc.scalar.activation(
                        gt[:, gb, :], ps[:], AF.Gelu_apprx_tanh,
                        bias=bfc[:, gb:gb + 1], scale=inv_f)

            # ---------------- mproj + residual ----------------
            with (
                tc.tile_pool(name="yout", bufs=3) as yop,
                tc.tile_pool(name="mo", bufs=3) as mop,
                tc.tile_pool(name="ps_m", bufs=4, space="PSUM") as psm,
            ):
                for tb in range(c.TB):
                    tbs = slice(tb * 128, (tb + 1) * 128)
                    yo = yop.tile([128, c.D], F32, tag="yo")
                    pss = [psm.tile([128, 512], F32, tag="ps", name=f"psm{q}")
                           for q in range(2)]
                    for g in range(NG):
                        gts = (gt[:, 2 * g:2 * g + 2, tbs]
                               if c.mproj_fp8 else gt[:, g, tbs])
                        for fh in range(2):
                            wms = (wm_all[fh][:, 2 * g:2 * g + 2, :]
                                   if c.mproj_fp8 else wm_all[fh][:, g, :])
                            nc.tensor.matmul(
                                pss[fh][:], gts, wms,
                                start=(g == 0), stop=(g == NG - 1),
                                perf_mode=PMm)
                    for fh in range(2):
                        fsl = slice(fh * 512, (fh + 1) * 512)
                        if c.mproj_fp8:
                            mo = mop.tile([128, 512], F32, tag="mo")
                            nc.scalar.activation(mo[:], pss[fh][:],
                                                 AF.Identity, scale=inv_m)
                            nc.vector.tensor_add(yo[:, fsl], mo[:],
                                                 x2b[tb][:, fsl])
                        else:
                            nc.vector.tensor_add(yo[:, fsl], pss[fh][:],
                                                 x2b[tb][:, fsl])
                    nc.sync.dma_start(
                        out=y_out[tb * 128:(tb + 1) * 128, :], in_=yo[:])

        es_x2.close()

    nc.compile()
    return nc


def core_rows(cfg, half):
    """absolute sequence rows owned by a core with parity half"""
    c = cfg
    loc = np.arange(c.T)
    return (2 * (loc // c.BS) + half) * c.BS + loc % c.BS


def make_core_inputs(cfg: Cfg, x, ln1_w, ln1_b, W_attn, b_attn, W_cproj,
                     b_cproj, ln2_w, ln2_b, W_fc, b_fc, W_mproj, b_mproj):
    """Split full inputs into one in_map per core."""
    c = cfg
    f32 = np.float32
    qt = ml_dtypes.float8_e4m3fn if c.qkv_fp8 else ml_dtypes.bfloat16

    # fold LN1 affine + query scale into W_attn / b_attn
    ln1_w = np.asarray(ln1_w, f32)
    ln1_b = np.asarray(ln1_b, f32)
    Wa = np.asarray(W_attn, f32) * ln1_w[:, None]
    ba = np.asarray(b_attn, f32) + ln1_b @ np.asarray(W_attn, f32)
    qs = 1.0 / math.sqrt(c.HD)
    Wa = Wa.copy()
    Wa[:, :c.D] *= qs
    ba = ba.copy()
    ba[:c.D] *= qs
    Wa_dev = (Wa * c.wscale).astype(qt)

    # fold LN2 affine into W_fc / b_fc
    ln2_w = np.asarray(ln2_w, f32)
    ln2_b = np.asarray(ln2_b, f32)
    Wf = np.asarray(W_fc, f32) * ln2_w[:, None]
    bf = np.asarray(b_fc, f32) + ln2_b @ np.asarray(W_fc, f32)

    fc_dt = ml_dtypes.float8_e4m3fn if c.fc_fp8 else ml_dtypes.bfloat16
    mp_dt = ml_dtypes.float8_e4m3fn if c.mproj_fp8 else ml_dtypes.bfloat16
    shared = {
        "w_attn": np.ascontiguousarray(Wa_dev),
        "w_cproj": np.ascontiguousarray(W_cproj).astype(ml_dtypes.bfloat16),
        "w_fc": np.ascontiguousarray(Wf * c.fscale).astype(fc_dt),
        "w_mproj": np.ascontiguousarray(
            np.asarray(W_mproj, f32) * c.mscale).astype(mp_dt),
        "bv": np.ascontiguousarray(ba[2 * c.D:3 * c.D]).reshape(1, c.D),
        "bcp": np.ascontiguousarray(b_cproj, f32).reshape(1, c.D),
        "bmp": np.ascontiguousarray(b_mproj, f32).reshape(1, c.D),
        "bfc": np.ascontiguousarray(bf.reshape(c.GB, 128).T),
        "battn_qk": np.ascontiguousarray(
            ba[:2 * c.D].reshape(2 * c.DC, 128).T),
    }

    x = np.asarray(x, f32)
    in_maps = []
    for core in range(c.n_cores):
        b, half = core // 2, core % 2
        own = core_rows(c, half)
        peer = core_rows(c, 1 - half)
        perm = np.concatenate([own, peer])
        m = dict(shared)
        m["x"] = np.ascontiguousarray(x[b][perm])
        m["qidx"] = own.astype(f32).reshape(1, c.T)
        kofs = np.empty((128, c.KC), f32)
        for kc in range(c.KC):
            kofs[:, kc] = perm[kc * 128 + np.arange(128)]
        m["kofs"] = kofs
        in_maps.append(m)
    return in_maps


_NC_CACHE = {}


def get_nc(cfg: Cfg):
    key = (cfg.B, cfg.S, cfg.D, cfg.H, cfg.F, cfg.qkv_fp8, cfg.fc_fp8,
           cfg.mproj_fp8, cfg.av_fp8, cfg.qk_fp8, cfg.BS)
    if key not in _NC_CACHE:
        _NC_CACHE[key] = build(cfg)
    return _NC_CACHE[key]


def kernel(**inputs) -> np.ndarray:
    from concourse.bass_utils import run_bass_kernel_spmd

    cfg = Cfg()
    nc = get_nc(cfg)
    in_maps = make_core_inputs(cfg, **inputs)
    res = run_bass_kernel_spmd(nc, in_maps, core_ids=list(range(cfg.n_cores)))
    B, S, D = cfg.B, cfg.S, cfg.D
    out = np.empty((B, S, D), np.float32)
    for core in range(cfg.n_cores):
        b, half = core // 2, core % 2
        out[b, core_rows(cfg, half), :] = res.results[core]["y"]
    return out


# revision 49
# speedup vs baseline: 1.0451x; 1.0451x over previous
"""Single transformer block on 8 NeuronCores — collective-free.

Sharding: core c = (batch b=c//2, parity p=c%2). Each core receives the FULL
sequence of its batch, permuted to [own-stripe | peer-stripe] order, and
recomputes K and V for all 2048 tokens locally — cheaper than the pairwise
AllGather it replaces (~55us extra PE vs ~270us of collective time) and it
deletes all DRAM bounce traffic.  Q / attention / c_proj / MLP cover only the
core's 1024 own (striped) tokens.

Tricks:
  - LayerNorm affine (w, b) folds host-side into the following matmul
    weights/bias, so on-chip LN is just (x - mean) * rsqrt(var + eps).
  - The 1/sqrt(hd) query scale folds host-side into W_q / b_q.
  - V is built directly in [128 key, KC, H, 65] layout with a ones column at
    65, so AV yields the softmax denominator for free and per-head V slices
    are zero-copy views.
  - Scores are computed transposed S^T[k, q]; causal mask is a 0/1 multiply
    on P = exp(S) (finite, exact).  A mask differs from all-ones only in the
    chunk's diagonal 128-query slot, so masks are [128, 128] and cheap.
  - AV accumulates a whole 512-query group into one [65, 512] PSUM bank;
    a key chunk whose minimal covered slot is s_min only runs over query
    columns >= s_min*BS, so late chunks run narrow (128-token stripes keep
    the causal waste small and both parities balanced).
  - fp8 (e4m3): QKV projections and mproj run DoubleRow matmuls (2x PE
    rate, half the instructions; weights pre-scaled x64 host-side to stay
    out of the fp8 subnormal range, undone at eviction).  P and V are fp8
    too (scores have std ~0.4, so P <= e^2.5 fits easily), which makes AV a
    DoubleRow over (own, peer) chunk pairs.  fc stays bf16: quantizing it
    pushed rel err past the gate (2.4e-2); this config measures 1.68e-2.
  - Both heads of a 128-feature block run as interleaved QK->exp->AV chains
    to hide cross-engine latency.
"""

import math
from contextlib import ExitStack

import numpy as np
import ml_dtypes

import concourse.bacc as bacc
import concourse.bass as bass
import concourse.mybir as mybir
import concourse.tile as tile
from concourse.masks import make_identity

F32 = mybir.dt.float32
F32R = mybir.dt.float32r
BF16 = mybir.dt.bfloat16
F8 = mybir.dt.float8e4
AF = mybir.ActivationFunctionType
ALU = mybir.AluOpType

EPS = 1e-5


class Cfg:
    def __init__(self, B=4, S=2048, D=1024, H=16, F=4096, n_cores=8,
                 qkv_fp8=True, fc_fp8=False, mproj_fp8=True, av_fp8=True,
                 qk_fp8=False, BS=128):
        self.B, self.S, self.D, self.H, self.F = B, S, D, H, F
        self.n_cores = n_cores
        assert n_cores == 2 * B
        self.HD = D // H
        assert self.HD == 64
        self.T = S // 2            # tokens owned per core
        self.KT = S // 128         # token 128-tiles, full sequence
        self.TB = self.T // 128    # token 128-tiles, local
        self.DC = D // 128         # contraction chunks over D
        self.QF = 512              # free-dim tile for projection matmuls
        self.KC = S // 128         # key 128-chunks over full sequence
        self.GB = F // 128         # MLP hidden 128-blocks
        self.HPB = 128 // self.HD  # heads per 128-feature block (=2)
        self.BS = BS               # stripe block (q-slot) size
        self.SLOTS = self.T // self.BS
        self.SPG = 512 // self.BS  # slots per 512-wide attention group
        self.KCH = self.KC // 2    # chunks per parity half
        self.CPB = self.BS // 128  # key chunks per stripe block
        self.qkv_fp8 = qkv_fp8
        self.fc_fp8 = fc_fp8
        self.mproj_fp8 = mproj_fp8
        self.av_fp8 = av_fp8
        self.qk_fp8 = qk_fp8
        self.wscale = 64.0 if qkv_fp8 else 1.0
        self.fscale = 64.0 if fc_fp8 else 1.0
        self.mscale = 64.0 if mproj_fp8 else 1.0


def build(cfg: Cfg):
    c = cfg
    QT = F8 if c.qkv_fp8 else BF16
    FT = F8 if c.fc_fp8 else BF16
    MPT = F8 if c.mproj_fp8 else BF16
    VT = F8 if c.av_fp8 else BF16
    PT = VT
    KQT = F8 if c.qk_fp8 else BF16
    nc = bacc.Bacc(None, target_bir_lowering=False)

    # ---------------- I/O ----------------
    x_in = nc.dram_tensor("x", [c.S, c.D], F32, kind="ExternalInput")
    w_attn = nc.dram_tensor("w_attn", [c.D, 3 * c.D], QT, kind="ExternalInput")
    w_cproj = nc.dram_tensor("w_cproj", [c.D, c.D], BF16, kind="ExternalInput")
    w_fc = nc.dram_tensor("w_fc", [c.D, c.F], FT, kind="ExternalInput")
    w_mproj = nc.dram_tensor("w_mproj", [c.F, c.D], MPT,
                             kind="ExternalInput")
    battn_qk_in = nc.dram_tensor("battn_qk", [128, 2 * c.DC], F32,
                                 kind="ExternalInput")
    bv_in = nc.dram_tensor("bv", [1, c.D], F32, kind="ExternalInput")
    bcp_in = nc.dram_tensor("bcp", [1, c.D], F32, kind="ExternalInput")
    bmp_in = nc.dram_tensor("bmp", [1, c.D], F32, kind="ExternalInput")
    bfc_in = nc.dram_tensor("bfc", [128, c.GB], F32, kind="ExternalInput")
    qidx_in = nc.dram_tensor("qidx", [1, c.T], F32, kind="ExternalInput")
    kofs_in = nc.dram_tensor("kofs", [128, c.KC], F32, kind="ExternalInput")
    y_out = nc.dram_tensor("y", [c.T, c.D], F32, kind="ExternalOutput")

    def bcast(dram, p=128):
        # partition-broadcast DMA source: read row 0 for every partition
        return bass.AP(tensor=dram, offset=0, ap=[[0, p], [1, dram.shape[1]]])

    with tile.TileContext(nc) as tc, ExitStack() as es:
        gconst = es.enter_context(tc.tile_pool(name="gconst", bufs=1))
        ident = gconst.tile([128, 128], F32)
        make_identity(nc, ident[:])
        eps_t = gconst.tile([128, 1], F32)
        nc.vector.memset(eps_t[:], EPS)
        ones64_f = gconst.tile([1, 64], F32)
        nc.vector.memset(ones64_f[:], 1.0)
        ones64 = gconst.tile([1, 64], F32R)
        nc.vector.tensor_copy(ones64[:], ones64_f[:])

        def layernorm_to(get_src, n_tiles, dest, lnp, ps_tr, tag,
                         interleave=None):
            """normalize token tiles and write feature-major into dest
            [128, DC, n_tiles*128].  get_src(tb) -> token-major [128, D] tile.
            interleave(g) is called after every 4th tile to emit consumer
            work early (keeps PE fed in emission order)."""
            for tb in range(n_tiles):
                src = get_src(tb)
                st = lnp.tile([128, 2, 6], F32, tag=f"{tag}st")
                for sg in range(2):
                    nc.vector.bn_stats(
                        out=st[:, sg, :], in_=src[:, sg * 512:(sg + 1) * 512])
                mv = lnp.tile([128, 2], F32, tag=f"{tag}mv")
                nc.vector.bn_aggr(out=mv[:], in_=st[:])
                sd = lnp.tile([128, 1], F32, tag=f"{tag}sd")
                nc.scalar.activation(sd[:], mv[:, 1:2], AF.Sqrt,
                                     bias=eps_t[:, 0:1])
                rs = lnp.tile([128, 1], F32, tag=f"{tag}rs")
                nc.vector.reciprocal(rs[:], sd[:])
                nrm = lnp.tile([128, c.D], F32, tag=f"{tag}n")
                nc.vector.tensor_scalar(
                    out=nrm[:], in0=src[:], scalar1=mv[:, 0:1],
                    scalar2=rs[:, 0:1], op0=ALU.subtract, op1=ALU.mult)
                for i2 in range(c.DC // 4):
                    pt = ps_tr.tile([128, 512], F32, tag=f"{tag}tr")
                    for j in range(4):
                        ch = 4 * i2 + j
                        nc.tensor.matmul(
                            pt[:, j * 128:(j + 1) * 128],
                            nrm[:, ch * 128:(ch + 1) * 128], ident[:],
                            is_transpose=True, start=(j == 0), stop=(j == 3))
                    nc.scalar.activation(
                        dest[:, 4 * i2:4 * i2 + 4, tb * 128:(tb + 1) * 128],
                        pt[:], AF.Identity)
                if interleave is not None and tb % 4 == 3:
                    interleave(tb // 4)

        # ---------------- persistent activations ----------------
        es_per = ExitStack()
        xloc = []
        xlp = es_per.enter_context(tc.tile_pool(name="xloc", bufs=1,
                                                side="left"))
        for tb in range(c.TB):
            t = xlp.tile([128, c.D], F32, tag=f"x{tb}", name=f"x{tb}")
            nc.sync.dma_start(out=t[:], in_=x_in[tb * 128:(tb + 1) * 128, :])
            xloc.append(t)

        es_kvq = ExitStack()
        kvqp = es_kvq.enter_context(tc.tile_pool(name="kvq", bufs=1,
                                                 side="right"))
        ktp = kvqp.tile([128, c.DC, c.S], KQT, name="ktp")
        vtt = kvqp.tile([128, 2, c.KCH, c.H, 65], VT, name="vtt")
        qtp = kvqp.tile([128, c.DC, c.T], KQT, name="qtp")
        nc.vector.memset(vtt[:, :, :, :, 64:65], 1.0)

        # ================= phase A: LN1 + QKV =================
        es_ht = ExitStack()
        htp = es_ht.enter_context(tc.tile_pool(name="htp", bufs=1))
        ht = htp.tile([128, c.DC, c.S], QT, name="ht")

        with (
            tc.tile_pool(name="aconst", bufs=1) as aconst,
            tc.tile_pool(name="xs", bufs=3) as xsp,
            tc.tile_pool(name="lnp", bufs=3) as lnp,
            tc.tile_pool(name="wa", bufs=2) as wap,
            tc.tile_pool(name="ko", bufs=4) as kop,
            tc.tile_pool(name="ps_tr", bufs=2, space="PSUM") as ps_tr,
            tc.tile_pool(name="ps_mm", bufs=2, space="PSUM") as psmm,
        ):
            battn_qk = aconst.tile([128, 2 * c.DC], F32)
            nc.sync.dma_start(out=battn_qk[:], in_=battn_qk_in[:, :])
            bv_b = aconst.tile([128, c.D], F32)
            nc.sync.dma_start(out=bv_b[:], in_=bcast(bv_in))
            bcp_b = aconst.tile([128, c.D], F32)
            nc.sync.dma_start(out=bcp_b[:], in_=bcast(bcp_in))

            wk = wap.tile([128, c.DC, c.D], QT, tag="wa", name="wk")
            nc.scalar.dma_start(
                out=wk[:],
                in_=w_attn[:, c.D:2 * c.D].rearrange("(i p) f -> p i f",
                                                     p=128))
            wv = wap.tile([128, c.DC, c.D], QT, tag="wa", name="wv")
            nc.scalar.dma_start(
                out=wv[:],
                in_=w_attn[:, 2 * c.D:3 * c.D].rearrange("(i p) f -> p i f",
                                                         p=128))

            inv_w = 1.0 / c.wscale
            NI = c.DC // 2 if c.qkv_fp8 else c.DC  # contraction steps

            def wsl(wslab, i, fsl):
                # weight slab contraction-step slice (pair of chunks in fp8)
                if c.qkv_fp8:
                    return wslab[:, 2 * i:2 * i + 2, fsl]
                return wslab[:, i, fsl]

            def hsl(i, tsl):
                if c.qkv_fp8:
                    return ht[:, 2 * i:2 * i + 2, tsl]
                return ht[:, i, tsl]

            PM = mybir.MatmulPerfMode.DoubleRow if c.qkv_fp8 else None

            def v_chunk(g):
                # V for these 4 token tiles; lhsT (ht slice) shared across
                # both feature halves so ldweights is loaded once per step
                for tb in range(4 * g, 4 * g + 4):
                    tbs = slice(tb * 128, (tb + 1) * 128)
                    pss = [psmm.tile([128, 512], F32, tag="ps", name=f"psv{q}")
                           for q in range(2)]
                    for i in range(NI):
                        for vh in range(2):
                            nc.tensor.matmul(
                                pss[vh][:], hsl(i, tbs),
                                wsl(wv, i, slice(vh * 512, (vh + 1) * 512)),
                                start=(i == 0), stop=(i == NI - 1),
                                perf_mode=PM)
                    for vh in range(2):
                        fsl = slice(vh * 512, (vh + 1) * 512)
                        dst = vtt[:, tb // c.KCH, tb % c.KCH,
                                  vh * 8:(vh + 1) * 8, 0:64]
                        if c.qkv_fp8:
                            vo = kop.tile([128, 512], BF16, tag="vo")
                            nc.scalar.activation(vo[:], pss[vh][:],
                                                 AF.Identity, scale=inv_w)
                            nc.vector.tensor_add(dst, vo[:], bv_b[:, fsl])
                        else:
                            nc.vector.tensor_add(dst, pss[vh][:],
                                                 bv_b[:, fsl])

            def get_src(tb):
                if tb < c.TB:
                    return xloc[tb]
                t = xsp.tile([128, c.D], F32, tag="xs")
                nc.sync.dma_start(out=t[:],
                                  in_=x_in[tb * 128:(tb + 1) * 128, :])
                return t

            layernorm_to(get_src, c.KT, ht, lnp, ps_tr, "a",
                         interleave=v_chunk)

            # ---- k^T pass: lhsT (w chunk) shared across 4 token slices ----
            for m in range(c.DC):
                msl = slice(m * 128, (m + 1) * 128)
                pss = [psmm.tile([128, 2, 512], F32, tag="pk",
                                 name=f"psk{q}") for q in range(2)]
                for i in range(NI):
                    for th in range(4):
                        nc.tensor.matmul(
                            pss[th // 2][:, th % 2, :], wsl(wk, i, msl),
                            hsl(i, slice(th * 512, (th + 1) * 512)),
                            start=(i == 0), stop=(i == NI - 1),
                            perf_mode=PM)
                for q in range(2):
                    nc.scalar.activation(
                        ktp[:, m, q * 1024:(q + 1) * 1024], pss[q][:],
                        AF.Identity,
                        bias=battn_qk[:, c.DC + m:c.DC + m + 1], scale=inv_w)

            # ---- q^T pass (local tokens only; scale folded host-side) ----
            wq = wap.tile([128, c.DC, c.D], QT, tag="wa", name="wq")
            nc.scalar.dma_start(
                out=wq[:],
                in_=w_attn[:, 0:c.D].rearrange("(i p) f -> p i f", p=128))
            for m in range(c.DC):
                msl = slice(m * 128, (m + 1) * 128)
                psq = psmm.tile([128, 2, 512], F32, tag="pk", name="psq")
                for i in range(NI):
                    for th in range(2):
                        nc.tensor.matmul(
                            psq[:, th, :], wsl(wq, i, msl),
                            hsl(i, slice(th * 512, (th + 1) * 512)),
                            start=(i == 0), stop=(i == NI - 1),
                            perf_mode=PM)
                nc.scalar.activation(
                    qtp[:, m, :], psq[:],
                    AF.Identity, bias=battn_qk[:, m:m + 1], scale=inv_w)

            # fold the c_proj bias into the residual copy of x, in place
            for tb in range(c.TB):
                nc.vector.tensor_add(xloc[tb][:], xloc[tb][:], bcp_b[:])

        es_ht.close()

        # ================= phase B: attention =================
        # prefetch c_proj weights during attention
        es_wc = ExitStack()
        wcp = es_wc.enter_context(tc.tile_pool(name="wc", bufs=1,
                                               side="left"))
        wc = wcp.tile([128, c.DC, c.D], BF16, name="wc")
        nc.scalar.dma_start(
            out=wc[:], in_=w_cproj[:, :].rearrange("(i p) f -> p i f", p=128))

        es_at = ExitStack()
        atp = es_at.enter_context(tc.tile_pool(name="atp", bufs=1,
                                               side="left"))
        at = atp.tile([128, c.DC, c.T], BF16, name="at")

        with (
            tc.tile_pool(name="bconst", bufs=1) as bconst,
            tc.tile_pool(name="mask", bufs=1) as maskp,
            tc.tile_pool(name="pt", bufs=6) as ptp,
            tc.tile_pool(name="rec", bufs=4) as recp,
            tc.tile_pool(name="ps_qk", bufs=2, space="PSUM") as psqk,
            tc.tile_pool(name="ps_o", bufs=3, space="PSUM") as pso,
            tc.tile_pool(name="ps_bc", bufs=1, space="PSUM") as psbc,
        ):
            qidx = bconst.tile([128, c.T], F32)
            nc.sync.dma_start(out=qidx[:], in_=bcast(qidx_in))
            kofs = bconst.tile([128, c.KC], F32)
            nc.sync.dma_start(out=kofs[:], in_=kofs_in[:, :])

            groups = [list(range(c.SPG * gi, c.SPG * (gi + 1)))
                      for gi in range(c.SLOTS // c.SPG)]

            # A chunk's mask differs from all-ones only in the diagonal
            # slot s_min (queries in later slots are >= every key of the
            # chunk for either parity), so one [128, BS] mask per chunk.
            masks = {}
            for kc in range(c.KC):
                s_min = (kc % c.KCH) // c.CPB
                qsl = slice(s_min * c.BS, (s_min + 1) * c.BS)
                mk = maskp.tile([128, c.BS], BF16, tag=f"mk{kc}",
                                name=f"mk{kc}")
                nc.vector.tensor_scalar(
                    out=mk[:], in0=qidx[:, qsl],
                    scalar1=kofs[:, kc:kc + 1], scalar2=None,
                    op0=ALU.is_ge)
                masks[kc] = mk

            # both heads of a feature block run interleaved: two independent
            # QK->exp->AV chains hide each other's cross-engine latencies
            for jj in range(c.DC):
                for gi, g in enumerate(groups):
                    s0, s3 = g[0], g[-1]
                    n_loc = (s3 + 1) * c.CPB
                    pos = [pso.tile([65, 512], F32, tag="po",
                                    name=f"po{hp}") for hp in range(c.HPB)]
                    for loc in range(n_loc):
                        lo = max(s0, loc // c.CPB)
                        w = (s3 - lo + 1) * c.BS
                        ocol = (lo - s0) * c.BS
                        qsl = slice(lo * c.BS, (s3 + 1) * c.BS)
                        for hp in range(c.HPB):
                            h = c.HPB * jj + hp
                            base = hp * 64
                            ps2 = psqk.tile([128, 2, 512], F32, tag="qk")
                            pt = ptp.tile([128, 2, 512], PT, tag="pt")
                            for ix in range(2):
                                kc = loc + ix * c.KCH
                                nc.tensor.matmul(
                                    ps2[:, ix, 0:w],
                                    ktp[base:base + 64, jj,
                                        kc * 128:(kc + 1) * 128],
                                    qtp[base:base + 64, jj, qsl],
                                    start=True, stop=True)
                            nc.scalar.activation(pt[:, :, 0:w],
                                                 ps2[:, :, 0:w], AF.Exp)
                            if loc // c.CPB >= s0:
                                for ix in range(2):
                                    kc = loc + ix * c.KCH
                                    nc.vector.tensor_mul(
                                        pt[:, ix, 0:c.BS],
                                        pt[:, ix, 0:c.BS], masks[kc][:])
                            if c.av_fp8:
                                nc.tensor.matmul(
                                    pos[hp][:, ocol:512],
                                    vtt[:, :, loc, h, :], pt[:, :, 0:w],
                                    start=(loc == 0),
                                    stop=(loc == n_loc - 1),
                                    perf_mode=mybir.MatmulPerfMode.DoubleRow)
                            else:
                                for ix in range(2):
                                    nc.tensor.matmul(
                                        pos[hp][:, ocol:512],
                                        vtt[:, ix, loc, h, :],
                                        pt[:, ix, 0:w],
                                        start=(loc == 0 and ix == 0),
                                        stop=(loc == n_loc - 1 and ix == 1))
                    # normalize by softmax denominator (row 64)
                    gq = slice(s0 * c.BS, s0 * c.BS + 512)
                    for hp in range(c.HPB):
                        base = hp * 64
                        po = pos[hp]
                        rec = recp.tile([1, 512], F32R, tag="rec")
                        with nc.allow_low_precision(
                                reason="softmax denom in f32r"):
                            nc.vector.reciprocal(rec[:], po[64:65, :])
                        bc = psbc.tile([64, 512], F32, tag="bc")
                        nc.tensor.matmul(bc[:], ones64[:], rec[:],
                                         start=True, stop=True)
                        bcs = recp.tile([64, 512], F32, tag="bcs")
                        nc.vector.tensor_copy(bcs[:], bc[:])
                        nc.vector.tensor_mul(
                            at[base:base + 64, jj, gq], po[0:64, :], bcs[:])

        es_kvq.close()

        # ================= phase C: c_proj + residual =================
        es_x2 = ExitStack()
        x2p = es_x2.enter_context(tc.tile_pool(name="x2p", bufs=1,
                                               side="right"))
        x2t = []
        with tc.tile_pool(name="ps_c", bufs=4, space="PSUM") as psc:
            for tb in range(c.TB):
                x2 = x2p.tile([128, c.D], F32, tag=f"x2_{tb}",
                              name=f"x2_{tb}")
                pss = [psc.tile([128, 512], F32, tag="ps", name=f"psc{q}") for q in range(2)]
                for i in range(c.DC):
                    for fh in range(2):
                        nc.tensor.matmul(
                            pss[fh][:], at[:, i, tb * 128:(tb + 1) * 128],
                            wc[:, i, fh * 512:(fh + 1) * 512],
                            start=(i == 0), stop=(i == c.DC - 1))
                for fh in range(2):
                    fsl = slice(fh * 512, (fh + 1) * 512)
                    nc.vector.tensor_add(x2[:, fsl], pss[fh][:],
                                         xloc[tb][:, fsl])
                x2t.append(x2)

        es_at.close()
        es_wc.close()
        es_per.close()

        # ================= phase D: LN2 + MLP =================
        with (
            tc.tile_pool(name="dconst", bufs=1) as dconst,
            tc.tile_pool(name="gt", bufs=1) as gtp,
            tc.tile_pool(name="wm", bufs=1) as wmp,
        ):
            bmp_b = dconst.tile([128, c.D], F32)
            nc.sync.dma_start(out=bmp_b[:], in_=bcast(bmp_in))
            bfc = dconst.tile([128, c.GB], F32)
            nc.sync.dma_start(out=bfc[:], in_=bfc_in[:, :])

            PMf = mybir.MatmulPerfMode.DoubleRow if c.fc_fp8 else None
            PMm = mybir.MatmulPerfMode.DoubleRow if c.mproj_fp8 else None
            NI2 = c.DC // 2 if c.fc_fp8 else c.DC    # fc contraction steps
            NG = c.GB // 2 if c.mproj_fp8 else c.GB  # mproj contraction steps
            inv_f = 1.0 / c.fscale
            inv_m = 1.0 / c.mscale
            gt = gtp.tile([128, c.GB, c.T], MPT, name="gt")
            x2b = [gtp.tile([128, c.D], F32, tag=f"x2b{tb}",
                            name=f"x2b{tb}") for tb in range(c.TB)]
            # mproj weights prefetched during LN2/fc on the sync queue so
            # they don't serialize behind the fc slab loads (scalar queue)
            wm_all = []
            for fh in range(2):
                wm = wmp.tile([128, c.GB, 512], MPT, tag=f"wm{fh}",
                              name=f"wm{fh}")
                nc.sync.dma_start(
                    out=wm[:],
                    in_=w_mproj[:, fh * 512:(fh + 1) * 512].rearrange(
                        "(g p) f -> p g f", p=128))
                wm_all.append(wm)
            with (
                tc.tile_pool(name="mtp", bufs=1) as mtp,
                tc.tile_pool(name="lnp2", bufs=2) as lnp2,
                tc.tile_pool(name="wf", bufs=2) as wfp,
                tc.tile_pool(name="ps_tr2", bufs=3, space="PSUM") as ps_tr2,
                tc.tile_pool(name="ps_g", bufs=2, space="PSUM") as psg,
            ):
                mt = mtp.tile([128, c.DC, c.T], FT, name="mt")
                layernorm_to(lambda tb: x2t[tb], c.TB, mt, lnp2, ps_tr2, "d")
                for tb in range(c.TB):
                    nc.vector.tensor_add(x2b[tb][:], x2t[tb][:], bmp_b[:])

                def msl(i, tsl):
                    if c.fc_fp8:
                        return mt[:, 2 * i:2 * i + 2, tsl]
                    return mt[:, i, tsl]

                # ---------------- fc + gelu ----------------
                wf = None
                for gb in range(c.GB):
                    if gb % 4 == 0:
                        wf = wfp.tile([128, c.DC, 512], FT, tag="wf",
                                      name=f"wf{gb}")
                        j = gb // 4
                        nc.scalar.dma_start(
                            out=wf[:],
                            in_=w_fc[:, j * 512:(j + 1) * 512].rearrange(
                                "(i p) f -> p i f", p=128))
                    gl = (gb % 4) * 128
                    ps = psg.tile([128, 1024], F32, tag="ps")
                    for i in range(NI2):
                        wfs = (wf[:, 2 * i:2 * i + 2, gl:gl + 128]
                               if c.fc_fp8 else wf[:, i, gl:gl + 128])
                        for th in range(2):
                            nc.tensor.matmul(
                                ps[:, th * 512:(th + 1) * 512], wfs,
                                msl(i, slice(th * 512, (th + 1) * 512)),
                                start=(i == 0), stop=(i == NI2 - 1),
                                perf_mode=PMf)
                    nc.scalar.activation(
                        gt[:, gb, :], ps[:], AF.Gelu_apprx_tanh,
                        bias=bfc[:, gb:gb + 1], scale=inv_f)

            # ---------------- mproj + residual ----------------
            with (
                tc.tile_pool(name="yout", bufs=3) as yop,
                tc.tile_pool(name="mo", bufs=3) as mop,
                tc.tile_pool(name="ps_m", bufs=4, space="PSUM") as psm,
            ):
                for tb in range(c.TB):
                    tbs = slice(tb * 128, (tb + 1) * 128)
                    yo = yop.tile([128, c.D], F32, tag="yo")
                    pss = [psm.tile([128, 512], F32, tag="ps", name=f"psm{q}")
                           for q in range(2)]
                    for g in range(NG):
                        gts = (gt[:, 2 * g:2 * g + 2, tbs]
                               if c.mproj_fp8 else gt[:, g, tbs])
                        for fh in range(2):
                            wms = (wm_all[fh][:, 2 * g:2 * g + 2, :]
                                   if c.mproj_fp8 else wm_all[fh][:, g, :])
                            nc.tensor.matmul(
                                pss[fh][:], gts, wms,
                                start=(g == 0), stop=(g == NG - 1),
                                perf_mode=PMm)
                    for fh in range(2):
                        fsl = slice(fh * 512, (fh + 1) * 512)
                        if c.mproj_fp8:
                            mo = mop.tile([128, 512], F32, tag="mo")
                            nc.scalar.activation(mo[:], pss[fh][:],
                                                 AF.Identity, scale=inv_m)
                            nc.vector.tensor_add(yo[:, fsl], mo[:],
                                                 x2b[tb][:, fsl])
                        else:
                            nc.vector.tensor_add(yo[:, fsl], pss[fh][:],
                                                 x2b[tb][:, fsl])
                    nc.sync.dma_start(
                        out=y_out[tb * 128:(tb + 1) * 128, :], in_=yo[:])

        es_x2.close()

    nc.compile()
    return nc


def core_rows(cfg, half):
    """absolute sequence rows owned by a core with parity half"""
    c = cfg
    loc = np.arange(c.T)
    return (2 * (loc // c.BS) + half) * c.BS + loc % c.BS


def make_core_inputs(cfg: Cfg, x, ln1_w, ln1_b, W_attn, b_attn, W_cproj,
                     b_cproj, ln2_w, ln2_b, W_fc, b_fc, W_mproj, b_mproj):
    """Split full inputs into one in_map per core."""
    c = cfg
    f32 = np.float32
    qt = ml_dtypes.float8_e4m3fn if c.qkv_fp8 else ml_dtypes.bfloat16

    # fold LN1 affine + query scale into W_attn / b_attn
    ln1_w = np.asarray(ln1_w, f32)
    ln1_b = np.asarray(ln1_b, f32)
    Wa = np.asarray(W_attn, f32) * ln1_w[:, None]
    ba = np.asarray(b_attn, f32) + ln1_b @ np.asarray(W_attn, f32)
    qs = 1.0 / math.sqrt(c.HD)
    Wa = Wa.copy()
    Wa[:, :c.D] *= qs
    ba = ba.copy()
    ba[:c.D] *= qs
    Wa_dev = (Wa * c.wscale).astype(qt)

    # fold LN2 affine into W_fc / b_fc
    ln2_w = np.asarray(ln2_w, f32)
    ln2_b = np.asarray(ln2_b, f32)
    Wf = np.asarray(W_fc, f32) * ln2_w[:, None]
    bf = np.asarray(b_fc, f32) + ln2_b @ np.asarray(W_fc, f32)

    fc_dt = ml_dtypes.float8_e4m3fn if c.fc_fp8 else ml_dtypes.bfloat16
    mp_dt = ml_dtypes.float8_e4m3fn if c.mproj_fp8 else ml_dtypes.bfloat16
    shared = {
        "w_attn": np.ascontiguousarray(Wa_dev),
        "w_cproj": np.ascontiguousarray(W_cproj).astype(ml_dtypes.bfloat16),
        "w_fc": np.ascontiguousarray(Wf * c.fscale).astype(fc_dt),
        "w_mproj": np.ascontiguousarray(
            np.asarray(W_mproj, f32) * c.mscale).astype(mp_dt),
        "bv": np.ascontiguousarray(ba[2 * c.D:3 * c.D]).reshape(1, c.D),
        "bcp": np.ascontiguousarray(b_cproj, f32).reshape(1, c.D),
        "bmp": np.ascontiguousarray(b_mproj, f32).reshape(1, c.D),
        "bfc": np.ascontiguousarray(bf.reshape(c.GB, 128).T),
        "battn_qk": np.ascontiguousarray(
            ba[:2 * c.D].reshape(2 * c.DC, 128).T),
    }

    x = np.asarray(x, f32)
    in_maps = []
    for core in range(c.n_cores):
        b, half = core // 2, core % 2
        own = core_rows(c, half)
        peer = core_rows(c, 1 - half)
        perm = np.concatenate([own, peer])
        m = dict(shared)
        m["x"] = np.ascontiguousarray(x[b][perm])
        m["qidx"] = own.astype(f32).reshape(1, c.T)
        kofs = np.empty((128, c.KC), f32)
        for kc in range(c.KC):
            kofs[:, kc] = perm[kc * 128 + np.arange(128)]
        m["kofs"] = kofs
        in_maps.append(m)
    return in_maps


_NC_CACHE = {}


def get_nc(cfg: Cfg):
    key = (cfg.B, cfg.S, cfg.D, cfg.H, cfg.F, cfg.qkv_fp8, cfg.fc_fp8,
           cfg.mproj_fp8, cfg.av_fp8, cfg.qk_fp8, cfg.BS)
    if key not in _NC_CACHE:
        _NC_CACHE[key] = build(cfg)
    return _NC_CACHE[key]


def kernel(**inputs) -> np.ndarray:
    from concourse.bass_utils import run_bass_kernel_spmd

    cfg = Cfg()
    nc = get_nc(cfg)
    in_maps = make_core_inputs(cfg, **inputs)
    res = run_bass_kernel_spmd(nc, in_maps, core_ids=list(range(cfg.n_cores)))
    B, S, D = cfg.B, cfg.S, cfg.D
    out = np.empty((B, S, D), np.float32)
    for core in range(cfg.n_cores):
        b, half = core // 2, core % 2
        out[b, core_rows(cfg, half), :] = res.results[core]["y"]
    return out


# revision 50
# speedup vs baseline: 1.0623x; 1.0165x over previous
"""Single transformer block on 8 NeuronCores — collective-free.

Sharding: core c = (batch b=c//2, parity p=c%2). Each core receives the FULL
sequence of its batch, permuted to [own-stripe | peer-stripe] order, and
recomputes K and V for all 2048 tokens locally — cheaper than the pairwise
AllGather it replaces (~55us extra PE vs ~270us of collective time) and it
deletes all DRAM bounce traffic.  Q / attention / c_proj / MLP cover only the
core's 1024 own (striped) tokens.

Tricks:
  - LayerNorm affine (w, b) folds host-side into the following matmul
    weights/bias, so on-chip LN is just (x - mean) * rsqrt(var + eps).
  - The 1/sqrt(hd) query scale folds host-side into W_q / b_q.
  - V is built directly in [128 key, KC, H, 65] layout with a ones column at
    65, so AV yields the softmax denominator for free and per-head V slices
    are zero-copy views.
  - Scores are computed transposed S^T[k, q]; causal mask is a 0/1 multiply
    on P = exp(S) (finite, exact).  A mask differs from all-ones only in the
    chunk's diagonal 128-query slot, so masks are [128, 128] and cheap.
  - AV accumulates a whole 512-query group into one [65, 512] PSUM bank;
    a key chunk whose minimal covered slot is s_min only runs over query
    columns >= s_min*BS, so late chunks run narrow (128-token stripes keep
    the causal waste small and both parities balanced).
  - fp8 (e4m3): QKV projections and mproj run DoubleRow matmuls (2x PE
    rate, half the instructions; weights pre-scaled x64 host-side to stay
    out of the fp8 subnormal range, undone at eviction).  P and V are fp8
    too (scores have std ~0.4, so P <= e^2.5 fits easily), which makes AV a
    DoubleRow over (own, peer) chunk pairs.  fc stays bf16: quantizing it
    pushed rel err past the gate (2.4e-2); this config measures 1.68e-2.
  - Both heads of a 128-feature block run as interleaved QK->exp->AV chains
    to hide cross-engine latency.
"""

import math
from contextlib import ExitStack

import numpy as np
import ml_dtypes

import concourse.bacc as bacc
import concourse.bass as bass
import concourse.mybir as mybir
import concourse.tile as tile
from concourse.masks import make_identity

F32 = mybir.dt.float32
F32R = mybir.dt.float32r
BF16 = mybir.dt.bfloat16
F8 = mybir.dt.float8e4
AF = mybir.ActivationFunctionType
ALU = mybir.AluOpType

EPS = 1e-5


class Cfg:
    def __init__(self, B=4, S=2048, D=1024, H=16, F=4096, n_cores=8,
                 qkv_fp8=True, fc_fp8=False, mproj_fp8=True, av_fp8=True,
                 qk_fp8=False, BS=128):
        self.B, self.S, self.D, self.H, self.F = B, S, D, H, F
        self.n_cores = n_cores
        assert n_cores == 2 * B
        self.HD = D // H
        assert self.HD == 64
        self.T = S // 2            # tokens owned per core
        self.KT = S // 128         # token 128-tiles, full sequence
        self.TB = self.T // 128    # token 128-tiles, local
        self.DC = D // 128         # contraction chunks over D
        self.QF = 512              # free-dim tile for projection matmuls
        self.KC = S // 128         # key 128-chunks over full sequence
        self.GB = F // 128         # MLP hidden 128-blocks
        self.HPB = 128 // self.HD  # heads per 128-feature block (=2)
        self.BS = BS               # stripe block (q-slot) size
        self.SLOTS = self.T // self.BS
        self.SPG = 512 // self.BS  # slots per 512-wide attention group
        self.KCH = self.KC // 2    # chunks per parity half
        self.CPB = self.BS // 128  # key chunks per stripe block
        self.qkv_fp8 = qkv_fp8
        self.fc_fp8 = fc_fp8
        self.mproj_fp8 = mproj_fp8
        self.av_fp8 = av_fp8
        self.qk_fp8 = qk_fp8
        self.wscale = 64.0 if qkv_fp8 else 1.0
        self.fscale = 64.0 if fc_fp8 else 1.0
        self.mscale = 64.0 if mproj_fp8 else 1.0


def build(cfg: Cfg):
    c = cfg
    QT = F8 if c.qkv_fp8 else BF16
    FT = F8 if c.fc_fp8 else BF16
    MPT = F8 if c.mproj_fp8 else BF16
    VT = F8 if c.av_fp8 else BF16
    PT = VT
    KQT = F8 if c.qk_fp8 else BF16
    nc = bacc.Bacc(None, target_bir_lowering=False)

    # ---------------- I/O ----------------
    x_in = nc.dram_tensor("x", [c.S, c.D], F32, kind="ExternalInput")
    w_attn = nc.dram_tensor("w_attn", [c.D, 3 * c.D], QT, kind="ExternalInput")
    w_cproj = nc.dram_tensor("w_cproj", [c.D, c.D], BF16, kind="ExternalInput")
    w_fc = nc.dram_tensor("w_fc", [c.D, c.F], FT, kind="ExternalInput")
    w_mproj = nc.dram_tensor("w_mproj", [c.F, c.D], MPT,
                             kind="ExternalInput")
    battn_qk_in = nc.dram_tensor("battn_qk", [128, 2 * c.DC], F32,
                                 kind="ExternalInput")
    bv_in = nc.dram_tensor("bv", [1, c.D], F32, kind="ExternalInput")
    bcp_in = nc.dram_tensor("bcp", [1, c.D], F32, kind="ExternalInput")
    bmp_in = nc.dram_tensor("bmp", [1, c.D], F32, kind="ExternalInput")
    bfc_in = nc.dram_tensor("bfc", [128, c.GB], F32, kind="ExternalInput")
    qidx_in = nc.dram_tensor("qidx", [1, c.T], F32, kind="ExternalInput")
    kofs_in = nc.dram_tensor("kofs", [128, c.KC], F32, kind="ExternalInput")
    y_out = nc.dram_tensor("y", [c.T, c.D], F32, kind="ExternalOutput")

    def bcast(dram, p=128):
        # partition-broadcast DMA source: read row 0 for every partition
        return bass.AP(tensor=dram, offset=0, ap=[[0, p], [1, dram.shape[1]]])

    with tile.TileContext(nc) as tc, ExitStack() as es:
        gconst = es.enter_context(tc.tile_pool(name="gconst", bufs=1))
        ident = gconst.tile([128, 128], F32)
        make_identity(nc, ident[:])
        eps_t = gconst.tile([128, 1], F32)
        nc.vector.memset(eps_t[:], EPS)
        ones64_f = gconst.tile([1, 64], F32)
        nc.vector.memset(ones64_f[:], 1.0)
        ones64 = gconst.tile([1, 64], F32R)
        nc.vector.tensor_copy(ones64[:], ones64_f[:])

        def layernorm_to(get_src, n_tiles, dest, lnp, ps_tr, tag,
                         interleave=None):
            """normalize token tiles and write feature-major into dest
            [128, DC, n_tiles*128].  get_src(tb) -> token-major [128, D] tile.
            interleave(g) is called after every 4th tile to emit consumer
            work early (keeps PE fed in emission order)."""
            for tb in range(n_tiles):
                src = get_src(tb)
                st = lnp.tile([128, 2, 6], F32, tag=f"{tag}st")
                for sg in range(2):
                    nc.vector.bn_stats(
                        out=st[:, sg, :], in_=src[:, sg * 512:(sg + 1) * 512])
                mv = lnp.tile([128, 2], F32, tag=f"{tag}mv")
                nc.vector.bn_aggr(out=mv[:], in_=st[:])
                sd = lnp.tile([128, 1], F32, tag=f"{tag}sd")
                nc.scalar.activation(sd[:], mv[:, 1:2], AF.Sqrt,
                                     bias=eps_t[:, 0:1])
                rs = lnp.tile([128, 1], F32, tag=f"{tag}rs")
                nc.vector.reciprocal(rs[:], sd[:])
                nrm = lnp.tile([128, c.D], F32, tag=f"{tag}n")
                nc.vector.tensor_scalar(
                    out=nrm[:], in0=src[:], scalar1=mv[:, 0:1],
                    scalar2=rs[:, 0:1], op0=ALU.subtract, op1=ALU.mult)
                for i2 in range(c.DC // 4):
                    pt = ps_tr.tile([128, 512], F32, tag=f"{tag}tr")
                    for j in range(4):
                        ch = 4 * i2 + j
                        nc.tensor.matmul(
                            pt[:, j * 128:(j + 1) * 128],
                            nrm[:, ch * 128:(ch + 1) * 128], ident[:],
                            is_transpose=True, start=(j == 0), stop=(j == 3))
                    nc.scalar.activation(
                        dest[:, 4 * i2:4 * i2 + 4, tb * 128:(tb + 1) * 128],
                        pt[:], AF.Identity)
                if interleave is not None and tb % 4 == 3:
                    interleave(tb // 4)

        # ---------------- persistent activations ----------------
        es_per = ExitStack()
        xloc = []
        xlp = es_per.enter_context(tc.tile_pool(name="xloc", bufs=1,
                                                side="left"))
        for tb in range(c.TB):
            t = xlp.tile([128, c.D], F32, tag=f"x{tb}", name=f"x{tb}")
            nc.sync.dma_start(out=t[:], in_=x_in[tb * 128:(tb + 1) * 128, :])
            xloc.append(t)

        es_kvq = ExitStack()
        kvqp = es_kvq.enter_context(tc.tile_pool(name="kvq", bufs=1,
                                                 side="right"))
        ktp = kvqp.tile([128, c.DC, c.S], KQT, name="ktp")
        vtt = kvqp.tile([128, 2, c.KCH, c.H, 65], VT, name="vtt")
        qtp = kvqp.tile([128, c.DC, c.T], KQT, name="qtp")
        nc.vector.memset(vtt[:, :, :, :, 64:65], 1.0)

        # ================= phase A: LN1 + QKV =================
        es_ht = ExitStack()
        htp = es_ht.enter_context(tc.tile_pool(name="htp", bufs=1))
        ht = htp.tile([128, c.DC, c.S], QT, name="ht")

        with (
            tc.tile_pool(name="aconst", bufs=1) as aconst,
            tc.tile_pool(name="xs", bufs=4) as xsp,
            tc.tile_pool(name="lnp", bufs=4) as lnp,
            tc.tile_pool(name="wa", bufs=3) as wap,
            tc.tile_pool(name="ko", bufs=6) as kop,
            tc.tile_pool(name="ps_tr", bufs=2, space="PSUM") as ps_tr,
            tc.tile_pool(name="ps_mm", bufs=2, space="PSUM") as psmm,
        ):
            battn_qk = aconst.tile([128, 2 * c.DC], F32)
            nc.sync.dma_start(out=battn_qk[:], in_=battn_qk_in[:, :])
            bv_b = aconst.tile([128, c.D], F32)
            nc.sync.dma_start(out=bv_b[:], in_=bcast(bv_in))
            bcp_b = aconst.tile([128, c.D], F32)
            nc.sync.dma_start(out=bcp_b[:], in_=bcast(bcp_in))

            wk = wap.tile([128, c.DC, c.D], QT, tag="wa", name="wk")
            nc.scalar.dma_start(
                out=wk[:],
                in_=w_attn[:, c.D:2 * c.D].rearrange("(i p) f -> p i f",
                                                     p=128))
            wv = wap.tile([128, c.DC, c.D], QT, tag="wa", name="wv")
            nc.scalar.dma_start(
                out=wv[:],
                in_=w_attn[:, 2 * c.D:3 * c.D].rearrange("(i p) f -> p i f",
                                                         p=128))

            inv_w = 1.0 / c.wscale
            NI = c.DC // 2 if c.qkv_fp8 else c.DC  # contraction steps

            def wsl(wslab, i, fsl):
                # weight slab contraction-step slice (pair of chunks in fp8)
                if c.qkv_fp8:
                    return wslab[:, 2 * i:2 * i + 2, fsl]
                return wslab[:, i, fsl]

            def hsl(i, tsl):
                if c.qkv_fp8:
                    return ht[:, 2 * i:2 * i + 2, tsl]
                return ht[:, i, tsl]

            PM = mybir.MatmulPerfMode.DoubleRow if c.qkv_fp8 else None

            def v_chunk(g):
                # V for these 4 token tiles; lhsT (ht slice) shared across
                # both feature halves so ldweights is loaded once per step
                for tb in range(4 * g, 4 * g + 4):
                    tbs = slice(tb * 128, (tb + 1) * 128)
                    pss = [psmm.tile([128, 512], F32, tag="ps", name=f"psv{q}")
                           for q in range(2)]
                    for i in range(NI):
                        for vh in range(2):
                            nc.tensor.matmul(
                                pss[vh][:], hsl(i, tbs),
                                wsl(wv, i, slice(vh * 512, (vh + 1) * 512)),
                                start=(i == 0), stop=(i == NI - 1),
                                perf_mode=PM)
                    for vh in range(2):
                        fsl = slice(vh * 512, (vh + 1) * 512)
                        dst = vtt[:, tb // c.KCH, tb % c.KCH,
                                  vh * 8:(vh + 1) * 8, 0:64]
                        if c.qkv_fp8:
                            vo = kop.tile([128, 512], BF16, tag="vo")
                            nc.scalar.activation(vo[:], pss[vh][:],
                                                 AF.Identity, scale=inv_w)
                            nc.vector.tensor_add(dst, vo[:], bv_b[:, fsl])
                        else:
                            nc.vector.tensor_add(dst, pss[vh][:],
                                                 bv_b[:, fsl])

            def get_src(tb):
                if tb < c.TB:
                    return xloc[tb]
                t = xsp.tile([128, c.D], F32, tag="xs")
                nc.sync.dma_start(out=t[:],
                                  in_=x_in[tb * 128:(tb + 1) * 128, :])
                return t

            layernorm_to(get_src, c.KT, ht, lnp, ps_tr, "a",
                         interleave=v_chunk)

            # ---- k^T pass: lhsT (w chunk) shared across 4 token slices ----
            for m in range(c.DC):
                msl = slice(m * 128, (m + 1) * 128)
                pss = [psmm.tile([128, 2, 512], F32, tag="pk",
                                 name=f"psk{q}") for q in range(2)]
                for i in range(NI):
                    for th in range(4):
                        nc.tensor.matmul(
                            pss[th // 2][:, th % 2, :], wsl(wk, i, msl),
                            hsl(i, slice(th * 512, (th + 1) * 512)),
                            start=(i == 0), stop=(i == NI - 1),
                            perf_mode=PM)
                for q in range(2):
                    nc.scalar.activation(
                        ktp[:, m, q * 1024:(q + 1) * 1024], pss[q][:],
                        AF.Identity,
                        bias=battn_qk[:, c.DC + m:c.DC + m + 1], scale=inv_w)

            # ---- q^T pass (local tokens only; scale folded host-side) ----
            wq = wap.tile([128, c.DC, c.D], QT, tag="wa", name="wq")
            nc.scalar.dma_start(
                out=wq[:],
                in_=w_attn[:, 0:c.D].rearrange("(i p) f -> p i f", p=128))
            for m in range(c.DC):
                msl = slice(m * 128, (m + 1) * 128)
                psq = psmm.tile([128, 2, 512], F32, tag="pk", name="psq")
                for i in range(NI):
                    for th in range(2):
                        nc.tensor.matmul(
                            psq[:, th, :], wsl(wq, i, msl),
                            hsl(i, slice(th * 512, (th + 1) * 512)),
                            start=(i == 0), stop=(i == NI - 1),
                            perf_mode=PM)
                nc.scalar.activation(
                    qtp[:, m, :], psq[:],
                    AF.Identity, bias=battn_qk[:, m:m + 1], scale=inv_w)

            # fold the c_proj bias into the residual copy of x, in place
            for tb in range(c.TB):
                nc.vector.tensor_add(xloc[tb][:], xloc[tb][:], bcp_b[:])

        es_ht.close()

        # ================= phase B: attention =================
        # prefetch c_proj weights during attention
        es_wc = ExitStack()
        wcp = es_wc.enter_context(tc.tile_pool(name="wc", bufs=1,
                                               side="left"))
        wc = wcp.tile([128, c.DC, c.D], BF16, name="wc")
        nc.scalar.dma_start(
            out=wc[:], in_=w_cproj[:, :].rearrange("(i p) f -> p i f", p=128))

        es_at = ExitStack()
        atp = es_at.enter_context(tc.tile_pool(name="atp", bufs=1,
                                               side="left"))
        at = atp.tile([128, c.DC, c.T], BF16, name="at")

        with (
            tc.tile_pool(name="bconst", bufs=1) as bconst,
            tc.tile_pool(name="mask", bufs=1) as maskp,
            tc.tile_pool(name="pt", bufs=6) as ptp,
            tc.tile_pool(name="rec", bufs=4) as recp,
            tc.tile_pool(name="ps_qk", bufs=2, space="PSUM") as psqk,
            tc.tile_pool(name="ps_o", bufs=3, space="PSUM") as pso,
            tc.tile_pool(name="ps_bc", bufs=1, space="PSUM") as psbc,
        ):
            qidx = bconst.tile([128, c.T], F32)
            nc.sync.dma_start(out=qidx[:], in_=bcast(qidx_in))
            kofs = bconst.tile([128, c.KC], F32)
            nc.sync.dma_start(out=kofs[:], in_=kofs_in[:, :])

            groups = [list(range(c.SPG * gi, c.SPG * (gi + 1)))
                      for gi in range(c.SLOTS // c.SPG)]

            # A chunk's mask differs from all-ones only in the diagonal
            # slot s_min (queries in later slots are >= every key of the
            # chunk for either parity), so one [128, BS] mask per chunk.
            masks = {}
            for kc in range(c.KC):
                s_min = (kc % c.KCH) // c.CPB
                qsl = slice(s_min * c.BS, (s_min + 1) * c.BS)
                mk = maskp.tile([128, c.BS], BF16, tag=f"mk{kc}",
                                name=f"mk{kc}")
                nc.vector.tensor_scalar(
                    out=mk[:], in0=qidx[:, qsl],
                    scalar1=kofs[:, kc:kc + 1], scalar2=None,
                    op0=ALU.is_ge)
                masks[kc] = mk

            # both heads of a feature block run interleaved: two independent
            # QK->exp->AV chains hide each other's cross-engine latencies
            for jj in range(c.DC):
                for gi, g in enumerate(groups):
                    s0, s3 = g[0], g[-1]
                    n_loc = (s3 + 1) * c.CPB
                    pos = [pso.tile([65, 512], F32, tag="po",
                                    name=f"po{hp}") for hp in range(c.HPB)]
                    for loc in range(n_loc):
                        lo = max(s0, loc // c.CPB)
                        w = (s3 - lo + 1) * c.BS
                        ocol = (lo - s0) * c.BS
                        qsl = slice(lo * c.BS, (s3 + 1) * c.BS)
                        for hp in range(c.HPB):
                            h = c.HPB * jj + hp
                            base = hp * 64
                            ps2 = psqk.tile([128, 2, 512], F32, tag="qk")
                            pt = ptp.tile([128, 2, 512], PT, tag="pt")
                            for ix in range(2):
                                kc = loc + ix * c.KCH
                                nc.tensor.matmul(
                                    ps2[:, ix, 0:w],
                                    ktp[base:base + 64, jj,
                                        kc * 128:(kc + 1) * 128],
                                    qtp[base:base + 64, jj, qsl],
                                    start=True, stop=True)
                            nc.scalar.activation(pt[:, :, 0:w],
                                                 ps2[:, :, 0:w], AF.Exp)
                            if loc // c.CPB >= s0:
                                for ix in range(2):
                                    kc = loc + ix * c.KCH
                                    nc.vector.tensor_mul(
                                        pt[:, ix, 0:c.BS],
                                        pt[:, ix, 0:c.BS], masks[kc][:])
                            if c.av_fp8:
                                nc.tensor.matmul(
                                    pos[hp][:, ocol:512],
                                    vtt[:, :, loc, h, :], pt[:, :, 0:w],
                                    start=(loc == 0),
                                    stop=(loc == n_loc - 1),
                                    perf_mode=mybir.MatmulPerfMode.DoubleRow)
                            else:
                                for ix in range(2):
                                    nc.tensor.matmul(
                                        pos[hp][:, ocol:512],
                                        vtt[:, ix, loc, h, :],
                                        pt[:, ix, 0:w],
                                        start=(loc == 0 and ix == 0),
                                        stop=(loc == n_loc - 1 and ix == 1))
                    # normalize by softmax denominator (row 64)
                    gq = slice(s0 * c.BS, s0 * c.BS + 512)
                    for hp in range(c.HPB):
                        base = hp * 64
                        po = pos[hp]
                        rec = recp.tile([1, 512], F32R, tag="rec")
                        with nc.allow_low_precision(
                                reason="softmax denom in f32r"):
                            nc.vector.reciprocal(rec[:], po[64:65, :])
                        bc = psbc.tile([64, 512], F32, tag="bc")
                        nc.tensor.matmul(bc[:], ones64[:], rec[:],
                                         start=True, stop=True)
                        bcs = recp.tile([64, 512], F32, tag="bcs")
                        nc.vector.tensor_copy(bcs[:], bc[:])
                        nc.vector.tensor_mul(
                            at[base:base + 64, jj, gq], po[0:64, :], bcs[:])

        es_kvq.close()

        # ================= phase C: c_proj + residual =================
        es_x2 = ExitStack()
        x2p = es_x2.enter_context(tc.tile_pool(name="x2p", bufs=1,
                                               side="right"))
        x2t = []
        with tc.tile_pool(name="ps_c", bufs=4, space="PSUM") as psc:
            for tb in range(c.TB):
                x2 = x2p.tile([128, c.D], F32, tag=f"x2_{tb}",
                              name=f"x2_{tb}")
                pss = [psc.tile([128, 512], F32, tag="ps", name=f"psc{q}") for q in range(2)]
                for i in range(c.DC):
                    for fh in range(2):
                        nc.tensor.matmul(
                            pss[fh][:], at[:, i, tb * 128:(tb + 1) * 128],
                            wc[:, i, fh * 512:(fh + 1) * 512],
                            start=(i == 0), stop=(i == c.DC - 1))
                for fh in range(2):
                    fsl = slice(fh * 512, (fh + 1) * 512)
                    nc.vector.tensor_add(x2[:, fsl], pss[fh][:],
                                         xloc[tb][:, fsl])
                x2t.append(x2)

        es_at.close()
        es_wc.close()
        es_per.close()

        # ================= phase D: LN2 + MLP =================
        with (
            tc.tile_pool(name="dconst", bufs=1) as dconst,
            tc.tile_pool(name="gt", bufs=1) as gtp,
            tc.tile_pool(name="wm", bufs=1) as wmp,
        ):
            bmp_b = dconst.tile([128, c.D], F32)
            nc.sync.dma_start(out=bmp_b[:], in_=bcast(bmp_in))
            bfc = dconst.tile([128, c.GB], F32)
            nc.sync.dma_start(out=bfc[:], in_=bfc_in[:, :])

            PMf = mybir.MatmulPerfMode.DoubleRow if c.fc_fp8 else None
            PMm = mybir.MatmulPerfMode.DoubleRow if c.mproj_fp8 else None
            NI2 = c.DC // 2 if c.fc_fp8 else c.DC    # fc contraction steps
            NG = c.GB // 2 if c.mproj_fp8 else c.GB  # mproj contraction steps
            inv_f = 1.0 / c.fscale
            inv_m = 1.0 / c.mscale
            gt = gtp.tile([128, c.GB, c.T], MPT, name="gt")
            x2b = [gtp.tile([128, c.D], F32, tag=f"x2b{tb}",
                            name=f"x2b{tb}") for tb in range(c.TB)]
            # mproj weights prefetched during LN2/fc on the sync queue so
            # they don't serialize behind the fc slab loads (scalar queue)
            wm_all = []
            for fh in range(2):
                wm = wmp.tile([128, c.GB, 512], MPT, tag=f"wm{fh}",
                              name=f"wm{fh}")
                nc.sync.dma_start(
                    out=wm[:],
                    in_=w_mproj[:, fh * 512:(fh + 1) * 512].rearrange(
                        "(g p) f -> p g f", p=128))
                wm_all.append(wm)
            with (
                tc.tile_pool(name="mtp", bufs=1) as mtp,
                tc.tile_pool(name="lnp2", bufs=3) as lnp2,
                tc.tile_pool(name="wf", bufs=3) as wfp,
                tc.tile_pool(name="ps_tr2", bufs=3, space="PSUM") as ps_tr2,
                tc.tile_pool(name="ps_g", bufs=2, space="PSUM") as psg,
            ):
                mt = mtp.tile([128, c.DC, c.T], FT, name="mt")
                layernorm_to(lambda tb: x2t[tb], c.TB, mt, lnp2, ps_tr2, "d")
                for tb in range(c.TB):
                    nc.vector.tensor_add(x2b[tb][:], x2t[tb][:], bmp_b[:])

                def msl(i, tsl):
                    if c.fc_fp8:
                        return mt[:, 2 * i:2 * i + 2, tsl]
                    return mt[:, i, tsl]

                # ---------------- fc + gelu ----------------
                wf = None
                for gb in range(c.GB):
                    if gb % 4 == 0:
                        wf = wfp.tile([128, c.DC, 512], FT, tag="wf",
                                      name=f"wf{gb}")
                        j = gb // 4
                        nc.scalar.dma_start(
                            out=wf[:],
                            in_=w_fc[:, j * 512:(j + 1) * 512].rearrange(
                                "(i p) f -> p i f", p=128))
                    gl = (gb % 4) * 128
                    ps = psg.tile([128, 1024], F32, tag="ps")
                    for i in range(NI2):
                        wfs = (wf[:, 2 * i:2 * i + 2, gl:gl + 128]
                               if c.fc_fp8 else wf[:, i, gl:gl + 128])
                        for th in range(2):
                            nc.tensor.matmul(
                                ps[:, th * 512:(th + 1) * 512], wfs,
                                msl(i, slice(th * 512, (th + 1) * 512)),
                                start=(i == 0), stop=(i == NI2 - 1),
                                perf_mode=PMf)
                    nc.scalar.activation(
                        gt[:, gb, :], ps[:], AF.Gelu_apprx_tanh,
                        bias=bfc[:, gb:gb + 1], scale=inv_f)

            # ---------------- mproj + residual ----------------
            with (
                tc.tile_pool(name="yout", bufs=3) as yop,
                tc.tile_pool(name="mo", bufs=3) as mop,
                tc.tile_pool(name="ps_m", bufs=4, space="PSUM") as psm,
            ):
                for tb in range(c.TB):
                    tbs = slice(tb * 128, (tb + 1) * 128)
                    yo = yop.tile([128, c.D], F32, tag="yo")
                    pss = [psm.tile([128, 512], F32, tag="ps", name=f"psm{q}")
                           for q in range(2)]
                    for g in range(NG):
                        gts = (gt[:, 2 * g:2 * g + 2, tbs]
                               if c.mproj_fp8 else gt[:, g, tbs])
                        for fh in range(2):
                            wms = (wm_all[fh][:, 2 * g:2 * g + 2, :]
                                   if c.mproj_fp8 else wm_all[fh][:, g, :])
                            nc.tensor.matmul(
                                pss[fh][:], gts, wms,
                                start=(g == 0), stop=(g == NG - 1),
                                perf_mode=PMm)
                    for fh in range(2):
                        fsl = slice(fh * 512, (fh + 1) * 512)
                        if c.mproj_fp8:
                            mo = mop.tile([128, 512], F32, tag="mo")
                            nc.scalar.activation(mo[:], pss[fh][:],
                                                 AF.Identity, scale=inv_m)
                            nc.vector.tensor_add(yo[:, fsl], mo[:],
                                                 x2b[tb][:, fsl])
                        else:
                            nc.vector.tensor_add(yo[:, fsl], pss[fh][:],
                                                 x2b[tb][:, fsl])
                    nc.sync.dma_start(
                        out=y_out[tb * 128:(tb + 1) * 128, :], in_=yo[:])

        es_x2.close()

    nc.compile()
    return nc


def core_rows(cfg, half):
    """absolute sequence rows owned by a core with parity half"""
    c = cfg
    loc = np.arange(c.T)
    return (2 * (loc // c.BS) + half) * c.BS + loc % c.BS


def make_core_inputs(cfg: Cfg, x, ln1_w, ln1_b, W_attn, b_attn, W_cproj,
                     b_cproj, ln2_w, ln2_b, W_fc, b_fc, W_mproj, b_mproj):
    """Split full inputs into one in_map per core."""
    c = cfg
    f32 = np.float32
    qt = ml_dtypes.float8_e4m3fn if c.qkv_fp8 else ml_dtypes.bfloat16

    # fold LN1 affine + query scale into W_attn / b_attn
    ln1_w = np.asarray(ln1_w, f32)
    ln1_b = np.asarray(ln1_b, f32)
    Wa = np.asarray(W_attn, f32) * ln1_w[:, None]
    ba = np.asarray(b_attn, f32) + ln1_b @ np.asarray(W_attn, f32)
    qs = 1.0 / math.sqrt(c.HD)
    Wa = Wa.copy()
    Wa[:, :c.D] *= qs
    ba = ba.copy()
    ba[:c.D] *= qs
    Wa_dev = (Wa * c.wscale).astype(qt)

    # fold LN2 affine into W_fc / b_fc
    ln2_w = np.asarray(ln2_w, f32)
    ln2_b = np.asarray(ln2_b, f32)
    Wf = np.asarray(W_fc, f32) * ln2_w[:, None]
    bf = np.asarray(b_fc, f32) + ln2_b @ np.asarray(W_fc, f32)

    fc_dt = ml_dtypes.float8_e4m3fn if c.fc_fp8 else ml_dtypes.bfloat16
    mp_dt = ml_dtypes.float8_e4m3fn if c.mproj_fp8 else ml_dtypes.bfloat16
    shared = {
        "w_attn": np.ascontiguousarray(Wa_dev),
        "w_cproj": np.ascontiguousarray(W_cproj).astype(ml_dtypes.bfloat16),
        "w_fc": np.ascontiguousarray(Wf * c.fscale).astype(fc_dt),
        "w_mproj": np.ascontiguousarray(
            np.asarray(W_mproj, f32) * c.mscale).astype(mp_dt),
        "bv": np.ascontiguousarray(ba[2 * c.D:3 * c.D]).reshape(1, c.D),
        "bcp": np.ascontiguousarray(b_cproj, f32).reshape(1, c.D),
        "bmp": np.ascontiguousarray(b_mproj, f32).reshape(1, c.D),
        "bfc": np.ascontiguousarray(bf.reshape(c.GB, 128).T),
        "battn_qk": np.ascontiguousarray(
            ba[:2 * c.D].reshape(2 * c.DC, 128).T),
    }

    x = np.asarray(x, f32)
    in_maps = []
    for core in range(c.n_cores):
        b, half = core // 2, core % 2
        own = core_rows(c, half)
        peer = core_rows(c, 1 - half)
        perm = np.concatenate([own, peer])
        m = dict(shared)
        m["x"] = np.ascontiguousarray(x[b][perm])
        m["qidx"] = own.astype(f32).reshape(1, c.T)
        kofs = np.empty((128, c.KC), f32)
        for kc in range(c.KC):
            kofs[:, kc] = perm[kc * 128 + np.arange(128)]
        m["kofs"] = kofs
        in_maps.append(m)
    return in_maps


_NC_CACHE = {}


def get_nc(cfg: Cfg):
    key = (cfg.B, cfg.S, cfg.D, cfg.H, cfg.F, cfg.qkv_fp8, cfg.fc_fp8,
           cfg.mproj_fp8, cfg.av_fp8, cfg.qk_fp8, cfg.BS)
    if key not in _NC_CACHE:
        _NC_CACHE[key] = build(cfg)
    return _NC_CACHE[key]


def kernel(**inputs) -> np.ndarray:
    from concourse.bass_utils import run_bass_kernel_spmd

    cfg = Cfg()
    nc = get_nc(cfg)
    in_maps = make_core_inputs(cfg, **inputs)
    res = run_bass_kernel_spmd(nc, in_maps, core_ids=list(range(cfg.n_cores)))
    B, S, D = cfg.B, cfg.S, cfg.D
    out = np.empty((B, S, D), np.float32)
    for core in range(cfg.n_cores):
        b, half = core // 2, core % 2
        out[b, core_rows(cfg, half), :] = res.results[core]["y"]
    return out


# revision 51
# speedup vs baseline: 1.0646x; 1.0022x over previous
"""Single transformer block on 8 NeuronCores — collective-free.

Sharding: core c = (batch b=c//2, parity p=c%2). Each core receives the FULL
sequence of its batch, permuted to [own-stripe | peer-stripe] order, and
recomputes K and V for all 2048 tokens locally — cheaper than the pairwise
AllGather it replaces (~55us extra PE vs ~270us of collective time) and it
deletes all DRAM bounce traffic.  Q / attention / c_proj / MLP cover only the
core's 1024 own (striped) tokens.

Tricks:
  - LayerNorm affine (w, b) folds host-side into the following matmul
    weights/bias, so on-chip LN is just (x - mean) * rsqrt(var + eps).
  - The 1/sqrt(hd) query scale folds host-side into W_q / b_q.
  - V is built directly in [128 key, KC, H, 65] layout with a ones column at
    65, so AV yields the softmax denominator for free and per-head V slices
    are zero-copy views.
  - Scores are computed transposed S^T[k, q]; causal mask is a 0/1 multiply
    on P = exp(S) (finite, exact).  A mask differs from all-ones only in the
    chunk's diagonal 128-query slot, so masks are [128, 128] and cheap.
  - AV accumulates a whole 512-query group into one [65, 512] PSUM bank;
    a key chunk whose minimal covered slot is s_min only runs over query
    columns >= s_min*BS, so late chunks run narrow (128-token stripes keep
    the causal waste small and both parities balanced).
  - fp8 (e4m3): QKV projections and mproj run DoubleRow matmuls (2x PE
    rate, half the instructions; weights pre-scaled x64 host-side to stay
    out of the fp8 subnormal range, undone at eviction).  P and V are fp8
    too (scores have std ~0.4, so P <= e^2.5 fits easily), which makes AV a
    DoubleRow over (own, peer) chunk pairs.  fc stays bf16: quantizing it
    pushed rel err past the gate (2.4e-2); this config measures 1.68e-2.
  - Both heads of a 128-feature block run as interleaved QK->exp->AV chains
    to hide cross-engine latency.
"""

import math
from contextlib import ExitStack

import numpy as np
import ml_dtypes

import concourse.bacc as bacc
import concourse.bass as bass
import concourse.mybir as mybir
import concourse.tile as tile
from concourse.masks import make_identity

F32 = mybir.dt.float32
F32R = mybir.dt.float32r
BF16 = mybir.dt.bfloat16
F8 = mybir.dt.float8e4
AF = mybir.ActivationFunctionType
ALU = mybir.AluOpType

EPS = 1e-5


class Cfg:
    def __init__(self, B=4, S=2048, D=1024, H=16, F=4096, n_cores=8,
                 qkv_fp8=True, fc_fp8=False, mproj_fp8=True, av_fp8=True,
                 qk_fp8=False, BS=128):
        self.B, self.S, self.D, self.H, self.F = B, S, D, H, F
        self.n_cores = n_cores
        assert n_cores == 2 * B
        self.HD = D // H
        assert self.HD == 64
        self.T = S // 2            # tokens owned per core
        self.KT = S // 128         # token 128-tiles, full sequence
        self.TB = self.T // 128    # token 128-tiles, local
        self.DC = D // 128         # contraction chunks over D
        self.QF = 512              # free-dim tile for projection matmuls
        self.KC = S // 128         # key 128-chunks over full sequence
        self.GB = F // 128         # MLP hidden 128-blocks
        self.HPB = 128 // self.HD  # heads per 128-feature block (=2)
        self.BS = BS               # stripe block (q-slot) size
        self.SLOTS = self.T // self.BS
        self.SPG = 512 // self.BS  # slots per 512-wide attention group
        self.KCH = self.KC // 2    # chunks per parity half
        self.CPB = self.BS // 128  # key chunks per stripe block
        self.qkv_fp8 = qkv_fp8
        self.fc_fp8 = fc_fp8
        self.mproj_fp8 = mproj_fp8
        self.av_fp8 = av_fp8
        self.qk_fp8 = qk_fp8
        self.wscale = 64.0 if qkv_fp8 else 1.0
        self.fscale = 64.0 if fc_fp8 else 1.0
        self.mscale = 64.0 if mproj_fp8 else 1.0


def build(cfg: Cfg):
    c = cfg
    QT = F8 if c.qkv_fp8 else BF16
    FT = F8 if c.fc_fp8 else BF16
    MPT = F8 if c.mproj_fp8 else BF16
    VT = F8 if c.av_fp8 else BF16
    PT = VT
    KQT = F8 if c.qk_fp8 else BF16
    nc = bacc.Bacc(None, target_bir_lowering=False)

    # ---------------- I/O ----------------
    x_in = nc.dram_tensor("x", [c.S, c.D], F32, kind="ExternalInput")
    w_attn = nc.dram_tensor("w_attn", [c.D, 3 * c.D], QT, kind="ExternalInput")
    w_cproj = nc.dram_tensor("w_cproj", [c.D, c.D], BF16, kind="ExternalInput")
    w_fc = nc.dram_tensor("w_fc", [c.D, c.F], FT, kind="ExternalInput")
    w_mproj = nc.dram_tensor("w_mproj", [c.F, c.D], MPT,
                             kind="ExternalInput")
    battn_qk_in = nc.dram_tensor("battn_qk", [128, 2 * c.DC], F32,
                                 kind="ExternalInput")
    bv_in = nc.dram_tensor("bv", [1, c.D], F32, kind="ExternalInput")
    bcp_in = nc.dram_tensor("bcp", [1, c.D], F32, kind="ExternalInput")
    bmp_in = nc.dram_tensor("bmp", [1, c.D], F32, kind="ExternalInput")
    bfc_in = nc.dram_tensor("bfc", [128, c.GB], F32, kind="ExternalInput")
    qidx_in = nc.dram_tensor("qidx", [1, c.T], F32, kind="ExternalInput")
    kofs_in = nc.dram_tensor("kofs", [128, c.KC], F32, kind="ExternalInput")
    y_out = nc.dram_tensor("y", [c.T, c.D], F32, kind="ExternalOutput")

    def bcast(dram, p=128):
        # partition-broadcast DMA source: read row 0 for every partition
        return bass.AP(tensor=dram, offset=0, ap=[[0, p], [1, dram.shape[1]]])

    with tile.TileContext(nc) as tc, ExitStack() as es:
        gconst = es.enter_context(tc.tile_pool(name="gconst", bufs=1))
        ident = gconst.tile([128, 128], F32)
        make_identity(nc, ident[:])
        eps_t = gconst.tile([128, 1], F32)
        nc.vector.memset(eps_t[:], EPS)
        ones64_f = gconst.tile([1, 64], F32)
        nc.vector.memset(ones64_f[:], 1.0)
        ones64 = gconst.tile([1, 64], F32R)
        nc.vector.tensor_copy(ones64[:], ones64_f[:])

        def layernorm_to(get_src, n_tiles, dest, lnp, ps_tr, tag,
                         interleave=None):
            """normalize token tiles and write feature-major into dest
            [128, DC, n_tiles*128].  get_src(tb) -> token-major [128, D] tile.
            interleave(g) is called after every 4th tile to emit consumer
            work early (keeps PE fed in emission order)."""
            for tb in range(n_tiles):
                src = get_src(tb)
                st = lnp.tile([128, 2, 6], F32, tag=f"{tag}st")
                for sg in range(2):
                    nc.vector.bn_stats(
                        out=st[:, sg, :], in_=src[:, sg * 512:(sg + 1) * 512])
                mv = lnp.tile([128, 2], F32, tag=f"{tag}mv")
                nc.vector.bn_aggr(out=mv[:], in_=st[:])
                sd = lnp.tile([128, 1], F32, tag=f"{tag}sd")
                nc.scalar.activation(sd[:], mv[:, 1:2], AF.Sqrt,
                                     bias=eps_t[:, 0:1])
                rs = lnp.tile([128, 1], F32, tag=f"{tag}rs")
                nc.vector.reciprocal(rs[:], sd[:])
                nrm = lnp.tile([128, c.D], F32, tag=f"{tag}n")
                nc.vector.tensor_scalar(
                    out=nrm[:], in0=src[:], scalar1=mv[:, 0:1],
                    scalar2=rs[:, 0:1], op0=ALU.subtract, op1=ALU.mult)
                for i2 in range(c.DC // 4):
                    pt = ps_tr.tile([128, 512], F32, tag=f"{tag}tr")
                    for j in range(4):
                        ch = 4 * i2 + j
                        nc.tensor.matmul(
                            pt[:, j * 128:(j + 1) * 128],
                            nrm[:, ch * 128:(ch + 1) * 128], ident[:],
                            is_transpose=True, start=(j == 0), stop=(j == 3))
                    nc.scalar.activation(
                        dest[:, 4 * i2:4 * i2 + 4, tb * 128:(tb + 1) * 128],
                        pt[:], AF.Identity)
                if interleave is not None and tb % 4 == 3:
                    interleave(tb // 4)

        # ---------------- persistent activations ----------------
        es_per = ExitStack()
        xloc = []
        xlp = es_per.enter_context(tc.tile_pool(name="xloc", bufs=1,
                                                side="left"))
        for tb in range(c.TB):
            t = xlp.tile([128, c.D], F32, tag=f"x{tb}", name=f"x{tb}")
            nc.sync.dma_start(out=t[:], in_=x_in[tb * 128:(tb + 1) * 128, :])
            xloc.append(t)

        es_kvq = ExitStack()
        kvqp = es_kvq.enter_context(tc.tile_pool(name="kvq", bufs=1,
                                                 side="right"))
        ktp = kvqp.tile([128, c.DC, c.S], KQT, name="ktp")
        vtt = kvqp.tile([128, 2, c.KCH, c.H, 65], VT, name="vtt")
        qtp = kvqp.tile([128, c.DC, c.T], KQT, name="qtp")
        nc.vector.memset(vtt[:, :, :, :, 64:65], 1.0)

        # ================= phase A: LN1 + QKV =================
        es_ht = ExitStack()
        htp = es_ht.enter_context(tc.tile_pool(name="htp", bufs=1))
        ht = htp.tile([128, c.DC, c.S], QT, name="ht")

        with (
            tc.tile_pool(name="aconst", bufs=1) as aconst,
            tc.tile_pool(name="xs", bufs=5) as xsp,
            tc.tile_pool(name="lnp", bufs=4) as lnp,
            tc.tile_pool(name="wa", bufs=3) as wap,
            tc.tile_pool(name="ko", bufs=6) as kop,
            tc.tile_pool(name="ps_tr", bufs=2, space="PSUM") as ps_tr,
            tc.tile_pool(name="ps_mm", bufs=2, space="PSUM") as psmm,
        ):
            battn_qk = aconst.tile([128, 2 * c.DC], F32)
            nc.sync.dma_start(out=battn_qk[:], in_=battn_qk_in[:, :])
            bv_b = aconst.tile([128, c.D], F32)
            nc.sync.dma_start(out=bv_b[:], in_=bcast(bv_in))
            bcp_b = aconst.tile([128, c.D], F32)
            nc.sync.dma_start(out=bcp_b[:], in_=bcast(bcp_in))

            wk = wap.tile([128, c.DC, c.D], QT, tag="wa", name="wk")
            nc.scalar.dma_start(
                out=wk[:],
                in_=w_attn[:, c.D:2 * c.D].rearrange("(i p) f -> p i f",
                                                     p=128))
            wv = wap.tile([128, c.DC, c.D], QT, tag="wa", name="wv")
            nc.scalar.dma_start(
                out=wv[:],
                in_=w_attn[:, 2 * c.D:3 * c.D].rearrange("(i p) f -> p i f",
                                                         p=128))

            inv_w = 1.0 / c.wscale
            NI = c.DC // 2 if c.qkv_fp8 else c.DC  # contraction steps

            def wsl(wslab, i, fsl):
                # weight slab contraction-step slice (pair of chunks in fp8)
                if c.qkv_fp8:
                    return wslab[:, 2 * i:2 * i + 2, fsl]
                return wslab[:, i, fsl]

            def hsl(i, tsl):
                if c.qkv_fp8:
                    return ht[:, 2 * i:2 * i + 2, tsl]
                return ht[:, i, tsl]

            PM = mybir.MatmulPerfMode.DoubleRow if c.qkv_fp8 else None

            def v_chunk(g):
                # V for these 4 token tiles; lhsT (ht slice) shared across
                # both feature halves so ldweights is loaded once per step
                for tb in range(4 * g, 4 * g + 4):
                    tbs = slice(tb * 128, (tb + 1) * 128)
                    pss = [psmm.tile([128, 512], F32, tag="ps", name=f"psv{q}")
                           for q in range(2)]
                    for i in range(NI):
                        for vh in range(2):
                            nc.tensor.matmul(
                                pss[vh][:], hsl(i, tbs),
                                wsl(wv, i, slice(vh * 512, (vh + 1) * 512)),
                                start=(i == 0), stop=(i == NI - 1),
                                perf_mode=PM)
                    for vh in range(2):
                        fsl = slice(vh * 512, (vh + 1) * 512)
                        dst = vtt[:, tb // c.KCH, tb % c.KCH,
                                  vh * 8:(vh + 1) * 8, 0:64]
                        if c.qkv_fp8:
                            vo = kop.tile([128, 512], BF16, tag="vo")
                            nc.scalar.activation(vo[:], pss[vh][:],
                                                 AF.Identity, scale=inv_w)
                            nc.vector.tensor_add(dst, vo[:], bv_b[:, fsl])
                        else:
                            nc.vector.tensor_add(dst, pss[vh][:],
                                                 bv_b[:, fsl])

            def get_src(tb):
                if tb < c.TB:
                    return xloc[tb]
                t = xsp.tile([128, c.D], F32, tag="xs")
                nc.sync.dma_start(out=t[:],
                                  in_=x_in[tb * 128:(tb + 1) * 128, :])
                return t

            layernorm_to(get_src, c.KT, ht, lnp, ps_tr, "a",
                         interleave=v_chunk)

            # ---- k^T pass: lhsT (w chunk) shared across 4 token slices ----
            for m in range(c.DC):
                msl = slice(m * 128, (m + 1) * 128)
                pss = [psmm.tile([128, 2, 512], F32, tag="pk",
                                 name=f"psk{q}") for q in range(2)]
                for i in range(NI):
                    for th in range(4):
                        nc.tensor.matmul(
                            pss[th // 2][:, th % 2, :], wsl(wk, i, msl),
                            hsl(i, slice(th * 512, (th + 1) * 512)),
                            start=(i == 0), stop=(i == NI - 1),
                            perf_mode=PM)
                for q in range(2):
                    nc.scalar.activation(
                        ktp[:, m, q * 1024:(q + 1) * 1024], pss[q][:],
                        AF.Identity,
                        bias=battn_qk[:, c.DC + m:c.DC + m + 1], scale=inv_w)

            # ---- q^T pass (local tokens only; scale folded host-side) ----
            wq = wap.tile([128, c.DC, c.D], QT, tag="wa", name="wq")
            nc.scalar.dma_start(
                out=wq[:],
                in_=w_attn[:, 0:c.D].rearrange("(i p) f -> p i f", p=128))
            for m in range(c.DC):
                msl = slice(m * 128, (m + 1) * 128)
                psq = psmm.tile([128, 2, 512], F32, tag="pk", name="psq")
                for i in range(NI):
                    for th in range(2):
                        nc.tensor.matmul(
                            psq[:, th, :], wsl(wq, i, msl),
                            hsl(i, slice(th * 512, (th + 1) * 512)),
                            start=(i == 0), stop=(i == NI - 1),
                            perf_mode=PM)
                nc.scalar.activation(
                    qtp[:, m, :], psq[:],
                    AF.Identity, bias=battn_qk[:, m:m + 1], scale=inv_w)

            # fold the c_proj bias into the residual copy of x, in place
            for tb in range(c.TB):
                nc.vector.tensor_add(xloc[tb][:], xloc[tb][:], bcp_b[:])

        es_ht.close()

        # ================= phase B: attention =================
        # prefetch c_proj weights during attention
        es_wc = ExitStack()
        wcp = es_wc.enter_context(tc.tile_pool(name="wc", bufs=1,
                                               side="left"))
        wc = wcp.tile([128, c.DC, c.D], BF16, name="wc")
        nc.scalar.dma_start(
            out=wc[:], in_=w_cproj[:, :].rearrange("(i p) f -> p i f", p=128))

        es_at = ExitStack()
        atp = es_at.enter_context(tc.tile_pool(name="atp", bufs=1,
                                               side="left"))
        at = atp.tile([128, c.DC, c.T], BF16, name="at")

        with (
            tc.tile_pool(name="bconst", bufs=1) as bconst,
            tc.tile_pool(name="mask", bufs=1) as maskp,
            tc.tile_pool(name="pt", bufs=8) as ptp,
            tc.tile_pool(name="rec", bufs=6) as recp,
            tc.tile_pool(name="ps_qk", bufs=2, space="PSUM") as psqk,
            tc.tile_pool(name="ps_o", bufs=3, space="PSUM") as pso,
            tc.tile_pool(name="ps_bc", bufs=1, space="PSUM") as psbc,
        ):
            qidx = bconst.tile([128, c.T], F32)
            nc.sync.dma_start(out=qidx[:], in_=bcast(qidx_in))
            kofs = bconst.tile([128, c.KC], F32)
            nc.sync.dma_start(out=kofs[:], in_=kofs_in[:, :])

            groups = [list(range(c.SPG * gi, c.SPG * (gi + 1)))
                      for gi in range(c.SLOTS // c.SPG)]

            # A chunk's mask differs from all-ones only in the diagonal
            # slot s_min (queries in later slots are >= every key of the
            # chunk for either parity), so one [128, BS] mask per chunk.
            masks = {}
            for kc in range(c.KC):
                s_min = (kc % c.KCH) // c.CPB
                qsl = slice(s_min * c.BS, (s_min + 1) * c.BS)
                mk = maskp.tile([128, c.BS], BF16, tag=f"mk{kc}",
                                name=f"mk{kc}")
                nc.vector.tensor_scalar(
                    out=mk[:], in0=qidx[:, qsl],
                    scalar1=kofs[:, kc:kc + 1], scalar2=None,
                    op0=ALU.is_ge)
                masks[kc] = mk

            # both heads of a feature block run interleaved: two independent
            # QK->exp->AV chains hide each other's cross-engine latencies
            for jj in range(c.DC):
                for gi, g in enumerate(groups):
                    s0, s3 = g[0], g[-1]
                    n_loc = (s3 + 1) * c.CPB
                    pos = [pso.tile([65, 512], F32, tag="po",
                                    name=f"po{hp}") for hp in range(c.HPB)]
                    for loc in range(n_loc):
                        lo = max(s0, loc // c.CPB)
                        w = (s3 - lo + 1) * c.BS
                        ocol = (lo - s0) * c.BS
                        qsl = slice(lo * c.BS, (s3 + 1) * c.BS)
                        for hp in range(c.HPB):
                            h = c.HPB * jj + hp
                            base = hp * 64
                            ps2 = psqk.tile([128, 2, 512], F32, tag="qk")
                            pt = ptp.tile([128, 2, 512], PT, tag="pt")
                            for ix in range(2):
                                kc = loc + ix * c.KCH
                                nc.tensor.matmul(
                                    ps2[:, ix, 0:w],
                                    ktp[base:base + 64, jj,
                                        kc * 128:(kc + 1) * 128],
                                    qtp[base:base + 64, jj, qsl],
                                    start=True, stop=True)
                            nc.scalar.activation(pt[:, :, 0:w],
                                                 ps2[:, :, 0:w], AF.Exp)
                            if loc // c.CPB >= s0:
                                for ix in range(2):
                                    kc = loc + ix * c.KCH
                                    nc.vector.tensor_mul(
                                        pt[:, ix, 0:c.BS],
                                        pt[:, ix, 0:c.BS], masks[kc][:])
                            if c.av_fp8:
                                nc.tensor.matmul(
                                    pos[hp][:, ocol:512],
                                    vtt[:, :, loc, h, :], pt[:, :, 0:w],
                                    start=(loc == 0),
                                    stop=(loc == n_loc - 1),
                                    perf_mode=mybir.MatmulPerfMode.DoubleRow)
                            else:
                                for ix in range(2):
                                    nc.tensor.matmul(
                                        pos[hp][:, ocol:512],
                                        vtt[:, ix, loc, h, :],
                                        pt[:, ix, 0:w],
                                        start=(loc == 0 and ix == 0),
                                        stop=(loc == n_loc - 1 and ix == 1))
                    # normalize by softmax denominator (row 64)
                    gq = slice(s0 * c.BS, s0 * c.BS + 512)
                    for hp in range(c.HPB):
                        base = hp * 64
                        po = pos[hp]
                        rec = recp.tile([1, 512], F32R, tag="rec")
                        with nc.allow_low_precision(
                                reason="softmax denom in f32r"):
                            nc.vector.reciprocal(rec[:], po[64:65, :])
                        bc = psbc.tile([64, 512], F32, tag="bc")
                        nc.tensor.matmul(bc[:], ones64[:], rec[:],
                                         start=True, stop=True)
                        bcs = recp.tile([64, 512], F32, tag="bcs")
                        nc.vector.tensor_copy(bcs[:], bc[:])
                        nc.vector.tensor_mul(
                            at[base:base + 64, jj, gq], po[0:64, :], bcs[:])

        es_kvq.close()

        # ================= phase C: c_proj + residual =================
        es_x2 = ExitStack()
        x2p = es_x2.enter_context(tc.tile_pool(name="x2p", bufs=1,
                                               side="right"))
        x2t = []
        with tc.tile_pool(name="ps_c", bufs=4, space="PSUM") as psc:
            for tb in range(c.TB):
                x2 = x2p.tile([128, c.D], F32, tag=f"x2_{tb}",
                              name=f"x2_{tb}")
                pss = [psc.tile([128, 512], F32, tag="ps", name=f"psc{q}") for q in range(2)]
                for i in range(c.DC):
                    for fh in range(2):
                        nc.tensor.matmul(
                            pss[fh][:], at[:, i, tb * 128:(tb + 1) * 128],
                            wc[:, i, fh * 512:(fh + 1) * 512],
                            start=(i == 0), stop=(i == c.DC - 1))
                for fh in range(2):
                    fsl = slice(fh * 512, (fh + 1) * 512)
                    nc.vector.tensor_add(x2[:, fsl], pss[fh][:],
                                         xloc[tb][:, fsl])
                x2t.append(x2)

        es_at.close()
        es_wc.close()
        es_per.close()

        # ================= phase D: LN2 + MLP =================
        with (
            tc.tile_pool(name="dconst", bufs=1) as dconst,
            tc.tile_pool(name="gt", bufs=1) as gtp,
            tc.tile_pool(name="wm", bufs=1) as wmp,
        ):
            bmp_b = dconst.tile([128, c.D], F32)
            nc.sync.dma_start(out=bmp_b[:], in_=bcast(bmp_in))
            bfc = dconst.tile([128, c.GB], F32)
            nc.sync.dma_start(out=bfc[:], in_=bfc_in[:, :])

            PMf = mybir.MatmulPerfMode.DoubleRow if c.fc_fp8 else None
            PMm = mybir.MatmulPerfMode.DoubleRow if c.mproj_fp8 else None
            NI2 = c.DC // 2 if c.fc_fp8 else c.DC    # fc contraction steps
            NG = c.GB // 2 if c.mproj_fp8 else c.GB  # mproj contraction steps
            inv_f = 1.0 / c.fscale
            inv_m = 1.0 / c.mscale
            gt = gtp.tile([128, c.GB, c.T], MPT, name="gt")
            x2b = [gtp.tile([128, c.D], F32, tag=f"x2b{tb}",
                            name=f"x2b{tb}") for tb in range(c.TB)]
            # mproj weights prefetched during LN2/fc on the sync queue so
            # they don't serialize behind the fc slab loads (scalar queue)
            wm_all = []
            for fh in range(2):
                wm = wmp.tile([128, c.GB, 512], MPT, tag=f"wm{fh}",
                              name=f"wm{fh}")
                nc.sync.dma_start(
                    out=wm[:],
                    in_=w_mproj[:, fh * 512:(fh + 1) * 512].rearrange(
                        "(g p) f -> p g f", p=128))
                wm_all.append(wm)
            with (
                tc.tile_pool(name="mtp", bufs=1) as mtp,
                tc.tile_pool(name="lnp2", bufs=3) as lnp2,
                tc.tile_pool(name="wf", bufs=3) as wfp,
                tc.tile_pool(name="ps_tr2", bufs=3, space="PSUM") as ps_tr2,
                tc.tile_pool(name="ps_g", bufs=2, space="PSUM") as psg,
            ):
                mt = mtp.tile([128, c.DC, c.T], FT, name="mt")
                layernorm_to(lambda tb: x2t[tb], c.TB, mt, lnp2, ps_tr2, "d")
                for tb in range(c.TB):
                    nc.vector.tensor_add(x2b[tb][:], x2t[tb][:], bmp_b[:])

                def msl(i, tsl):
                    if c.fc_fp8:
                        return mt[:, 2 * i:2 * i + 2, tsl]
                    return mt[:, i, tsl]

                # ---------------- fc + gelu ----------------
                wf = None
                for gb in range(c.GB):
                    if gb % 4 == 0:
                        wf = wfp.tile([128, c.DC, 512], FT, tag="wf",
                                      name=f"wf{gb}")
                        j = gb // 4
                        nc.scalar.dma_start(
                            out=wf[:],
                            in_=w_fc[:, j * 512:(j + 1) * 512].rearrange(
                                "(i p) f -> p i f", p=128))
                    gl = (gb % 4) * 128
                    ps = psg.tile([128, 1024], F32, tag="ps")
                    for i in range(NI2):
                        wfs = (wf[:, 2 * i:2 * i + 2, gl:gl + 128]
                               if c.fc_fp8 else wf[:, i, gl:gl + 128])
                        for th in range(2):
                            nc.tensor.matmul(
                                ps[:, th * 512:(th + 1) * 512], wfs,
                                msl(i, slice(th * 512, (th + 1) * 512)),
                                start=(i == 0), stop=(i == NI2 - 1),
                                perf_mode=PMf)
                    nc.scalar.activation(
                        gt[:, gb, :], ps[:], AF.Gelu_apprx_tanh,
                        bias=bfc[:, gb:gb + 1], scale=inv_f)

            # ---------------- mproj + residual ----------------
            with (
                tc.tile_pool(name="yout", bufs=4) as yop,
                tc.tile_pool(name="mo", bufs=4) as mop,
                tc.tile_pool(name="ps_m", bufs=4, space="PSUM") as psm,
            ):
                for tb in range(c.TB):
                    tbs = slice(tb * 128, (tb + 1) * 128)
                    yo = yop.tile([128, c.D], F32, tag="yo")
                    pss = [psm.tile([128, 512], F32, tag="ps", name=f"psm{q}")
                           for q in range(2)]
                    for g in range(NG):
                        gts = (gt[:, 2 * g:2 * g + 2, tbs]
                               if c.mproj_fp8 else gt[:, g, tbs])
                        for fh in range(2):
                            wms = (wm_all[fh][:, 2 * g:2 * g + 2, :]
                                   if c.mproj_fp8 else wm_all[fh][:, g, :])
                            nc.tensor.matmul(
                                pss[fh][:], gts, wms,
                                start=(g == 0), stop=(g == NG - 1),
                                perf_mode=PMm)
                    for fh in range(2):
                        fsl = slice(fh * 512, (fh + 1) * 512)
                        if c.mproj_fp8:
                            mo = mop.tile([128, 512], F32, tag="mo")
                            nc.scalar.activation(mo[:], pss[fh][:],
                                                 AF.Identity, scale=inv_m)
                            nc.vector.tensor_add(yo[:, fsl], mo[:],
                                                 x2b[tb][:, fsl])
                        else:
                            nc.vector.tensor_add(yo[:, fsl], pss[fh][:],
                                                 x2b[tb][:, fsl])
                    nc.sync.dma_start(
                        out=y_out[tb * 128:(tb + 1) * 128, :], in_=yo[:])

        es_x2.close()

    nc.compile()
    return nc


def core_rows(cfg, half):
    """absolute sequence rows owned by a core with parity half"""
    c = cfg
    loc = np.arange(c.T)
    return (2 * (loc // c.BS) + half) * c.BS + loc % c.BS


def make_core_inputs(cfg: Cfg, x, ln1_w, ln1_b, W_attn, b_attn, W_cproj,
                     b_cproj, ln2_w, ln2_b, W_fc, b_fc, W_mproj, b_mproj):
    """Split full inputs into one in_map per core."""
    c = cfg
    f32 = np.float32
    qt = ml_dtypes.float8_e4m3fn if c.qkv_fp8 else ml_dtypes.bfloat16

    # fold LN1 affine + query scale into W_attn / b_attn
    ln1_w = np.asarray(ln1_w, f32)
    ln1_b = np.asarray(ln1_b, f32)
    Wa = np.asarray(W_attn, f32) * ln1_w[:, None]
    ba = np.asarray(b_attn, f32) + ln1_b @ np.asarray(W_attn, f32)
    qs = 1.0 / math.sqrt(c.HD)
    Wa = Wa.copy()
    Wa[:, :c.D] *= qs
    ba = ba.copy()
    ba[:c.D] *= qs
    Wa_dev = (Wa * c.wscale).astype(qt)

    # fold LN2 affine into W_fc / b_fc
    ln2_w = np.asarray(ln2_w, f32)
    ln2_b = np.asarray(ln2_b, f32)
    Wf = np.asarray(W_fc, f32) * ln2_w[:, None]
    bf = np.asarray(b_fc, f32) + ln2_b @ np.asarray(W_fc, f32)

    fc_dt = ml_dtypes.float8_e4m3fn if c.fc_fp8 else ml_dtypes.bfloat16
    mp_dt = ml_dtypes.float8_e4m3fn if c.mproj_fp8 else ml_dtypes.bfloat16
    shared = {
        "w_attn": np.ascontiguousarray(Wa_dev),
        "w_cproj": np.ascontiguousarray(W_cproj).astype(ml_dtypes.bfloat16),
        "w_fc": np.ascontiguousarray(Wf * c.fscale).astype(fc_dt),
        "w_mproj": np.ascontiguousarray(
            np.asarray(W_mproj, f32) * c.mscale).astype(mp_dt),
        "bv": np.ascontiguousarray(ba[2 * c.D:3 * c.D]).reshape(1, c.D),
        "bcp": np.ascontiguousarray(b_cproj, f32).reshape(1, c.D),
        "bmp": np.ascontiguousarray(b_mproj, f32).reshape(1, c.D),
        "bfc": np.ascontiguousarray(bf.reshape(c.GB, 128).T),
        "battn_qk": np.ascontiguousarray(
            ba[:2 * c.D].reshape(2 * c.DC, 128).T),
    }

    x = np.asarray(x, f32)
    in_maps = []
    for core in range(c.n_cores):
        b, half = core // 2, core % 2
        own = core_rows(c, half)
        peer = core_rows(c, 1 - half)
        perm = np.concatenate([own, peer])
        m = dict(shared)
        m["x"] = np.ascontiguousarray(x[b][perm])
        m["qidx"] = own.astype(f32).reshape(1, c.T)
        kofs = np.empty((128, c.KC), f32)
        for kc in range(c.KC):
            kofs[:, kc] = perm[kc * 128 + np.arange(128)]
        m["kofs"] = kofs
        in_maps.append(m)
    return in_maps


_NC_CACHE = {}


def get_nc(cfg: Cfg):
    key = (cfg.B, cfg.S, cfg.D, cfg.H, cfg.F, cfg.qkv_fp8, cfg.fc_fp8,
           cfg.mproj_fp8, cfg.av_fp8, cfg.qk_fp8, cfg.BS)
    if key not in _NC_CACHE:
        _NC_CACHE[key] = build(cfg)
    return _NC_CACHE[key]


def kernel(**inputs) -> np.ndarray:
    from concourse.bass_utils import run_bass_kernel_spmd

    cfg = Cfg()
    nc = get_nc(cfg)
    in_maps = make_core_inputs(cfg, **inputs)
    res = run_bass_kernel_spmd(nc, in_maps, core_ids=list(range(cfg.n_cores)))
    B, S, D = cfg.B, cfg.S, cfg.D
    out = np.empty((B, S, D), np.float32)
    for core in range(cfg.n_cores):
        b, half = core // 2, core % 2
        out[b, core_rows(cfg, half), :] = res.results[core]["y"]
    return out
